# revision 36
# baseline (speedup 1.0000x reference)
"""Additive attention (Bahdanau) Trainium2 Bass kernel.

Sharding: data-parallel over batch B=64 -> 8 cores x 8 batches.
Per core, batches processed in 2 groups of 4 (score rows live at PSUM
partitions {0,32,64,96}):
  proj[d, n] = W_img @ img[b, n, :]           PE, [d,n] tiles, lhsT=W_imgT
  tanh tiles = tanh(proj + ph[d])             ACT, per-partition bias fused
  score row [1, 512] = W_score . tanh         PE, static [128,1] lhsT (row b%4*32)
  group softmax: exp/sum/normalize            ACT+DVE on [128, 4096] group tile
  w columns for context via DMA-xbar transpose of bf16 weights
  context = sum_n w[n] * img[b, n, :]         PE, lhsT = w column [128,1]

Host feeds partition-tiled layouts so every DMA has 4 KiB contiguous runs:
  imgT_q [8, 128, 4, 4096]  (p, f-chunk, n)  proj rhs tiles
  imgN_q [8, 128, 32, 512]  (p, n-chunk, f)  context rhs tiles

TRN2 instructions embed only ONE sync wait; Bacc's generate_event_semaphores
legalizes the rest, and ops are arranged to keep waits rare. Emission order
software-pipelines context(group 0) against proj(group 1).
"""

import sys
import numpy as np

for p in ("/opt/trn_rl_repo",):
    if p not in sys.path:
        sys.path.insert(0, p)

import ml_dtypes

B, N, F, H, D = 64, 4096, 512, 512, 256
NCORES = 8
BPC = B // NCORES  # batches per core
NT = 2048          # n-tile size for proj loads
NNT = N // NT      # 2 tiles
NSUB = NT // 512   # 4 x 512 matmul slices per tile
NCH = N // 128     # 32 n-chunks
CG = 4             # context chunks per load
_BF = ml_dtypes.bfloat16

_nc_cache = {}


def _build_nc():
    if "nc" in _nc_cache:
        return _nc_cache["nc"]
    from contextlib import ExitStack

    import concourse.bass as bass  # noqa: F401
    import concourse.tile as tile
    from concourse import bacc, mybir

    bf16 = mybir.dt.bfloat16
    f32 = mybir.dt.float32
    AF = mybir.ActivationFunctionType

    nc = bacc.Bacc("TRN2")

    imgT = nc.declare_dram_parameter("imgT", [BPC, 128, 4, N], bf16, isOutput=False)
    imgN = nc.declare_dram_parameter("imgN", [BPC, 128, NCH, F], bf16, isOutput=False)
    # hw = [W_hid.T | hidden.T] packed: one DMA -> one wait on the f32 matmul
    hw = nc.declare_dram_parameter("hw", [H, D + BPC], f32, isOutput=False)
    wimgT = nc.declare_dram_parameter("wimgT", [F, D], bf16, isOutput=False)
    wscore = nc.declare_dram_parameter("wscore", [D, 1], bf16, isOutput=False)
    ident = nc.declare_dram_parameter("ident", [128, 128], bf16, isOutput=False)
    ctx_out = nc.declare_dram_parameter("ctx", [BPC, F], f32, isOutput=True)
    wts_out = nc.declare_dram_parameter("wts", [BPC, N], f32, isOutput=True)

    with tile.TileContext(nc) as tc, ExitStack() as ctx:
        const = ctx.enter_context(tc.tile_pool(name="const", bufs=1))
        itp = ctx.enter_context(tc.tile_pool(name="imgTp", bufs=2))
        inp = ctx.enter_context(tc.tile_pool(name="imgNp", bufs=3))
        thp = ctx.enter_context(tc.tile_pool(name="tanh", bufs=4))
        scrp = ctx.enter_context(tc.tile_pool(name="scrp", bufs=1))
        wtp = ctx.enter_context(tc.tile_pool(name="wtp", bufs=1))
        exfp = ctx.enter_context(tc.tile_pool(name="exfp", bufs=1))
        wnfp = ctx.enter_context(tc.tile_pool(name="wnfp", bufs=2))
        smp = ctx.enter_context(tc.tile_pool(name="smp", bufs=2))
        csp = ctx.enter_context(tc.tile_pool(name="csb", bufs=BPC))
        pps = ctx.enter_context(tc.tile_pool(name="pp", bufs=4, space="PSUM"))
        scp = ctx.enter_context(tc.tile_pool(name="sc", bufs=1, space="PSUM"))
        tpp = ctx.enter_context(tc.tile_pool(name="tp", bufs=2, space="PSUM"))
        ctxps = ctx.enter_context(tc.tile_pool(name="cps", bufs=1, space="PSUM"))

        # ---- constants (consumers arranged for <=1 wait each) ----
        wimg_sb = const.tile([128, 4, D], bf16)
        nc.sync.dma_start(wimg_sb[:], wimgT.rearrange("(c p) d -> p c d", p=128))
        hw_sb = const.tile([128, 4, D + BPC], f32)
        nc.sync.dma_start(hw_sb[:], hw.rearrange("(c p) x -> p c x", p=128))
        wsc_dma = const.tile([128, 2, 1], bf16)
        nc.sync.dma_start(wsc_dma[:], wscore.rearrange("(c p) o -> p c o", p=128))
        wsc_sb = const.tile([128, 2, 1], bf16)
        nc.scalar.copy(wsc_sb[:], wsc_dma[:])
        id_sb = const.tile([128, 128], bf16)
        nc.sync.dma_start(id_sb[:], ident[:])

        # ---- proj_hidden in [d, b] layout (per-partition bias for tanh) ----
        ph_ps = tpp.tile([128, 2, BPC], f32, tag="tp")
        for dh in range(2):
            for hc in range(4):
                nc.tensor.matmul(
                    ph_ps[:, dh, :],
                    lhsT=hw_sb[:, hc, dh * 128:(dh + 1) * 128],
                    rhs=hw_sb[:, hc, D:D + BPC],
                    start=(hc == 0),
                    stop=(hc == 3),
                )
        ph_sb = const.tile([128, 2, BPC], f32)
        nc.scalar.copy(ph_sb[:], ph_ps[:])

        sc_rows = {}   # group -> [128, 8, 512] f32 score rows (4 live rows)
        wn_bf = {}     # group -> [128, NCH, 128] bf16 transposed weights
        wn_fs = {}     # group -> [128, N] f32 normalized weights (for output)
        csbs = {}      # batch -> [1, F] f32 context rows (for output)

        def phase_a(b):
            """proj + tanh + score rows for one batch."""
            g, k = divmod(b, 4)
            if k == 0:
                srow_new = scrp.tile([128, NNT * NSUB, 512], f32, tag="scr")
                sc_rows[g] = srow_new
            srow = sc_rows[g]
            for nt in range(NNT):
                it = itp.tile([128, 4, NT], bf16, tag="it")
                nc.sync.dma_start(it[:], imgT[b, :, :, nt * NT:(nt + 1) * NT])
                for sub in range(NSUB):
                    sl = slice(sub * 512, (sub + 1) * 512)
                    ths = []
                    for dh in range(2):
                        pp = pps.tile([128, 512], f32, tag="pp")
                        for fc in range(4):
                            nc.tensor.matmul(
                                pp[:],
                                lhsT=wimg_sb[:, fc, dh * 128:(dh + 1) * 128],
                                rhs=it[:, fc, sl],
                                start=(fc == 0),
                                stop=(fc == 3),
                            )
                        th = thp.tile([128, 512], bf16, tag="th")
                        nc.scalar.activation(
                            th[:], pp[:], AF.Tanh,
                            bias=ph_sb[:, dh, b:b + 1], scale=1.0,
                        )
                        ths.append(th)
                    sc = scp.tile([128, 512], f32, tag="sc")
                    r = 32 * k
                    for dh in range(2):
                        nc.tensor.matmul(
                            sc[r:r + 1, :],
                            lhsT=wsc_sb[:, dh, :],
                            rhs=ths[dh][:],
                            start=(dh == 0),
                            stop=(dh == 1),
                            tile_position=(0, r),
                        )
                    nc.vector.tensor_copy(
                        srow[r:r + 1, nt * NSUB + sub, :], sc[r:r + 1, :]
                    )

        def softmax_group(g):
            """exp/sum/normalize/transpose for 4 batches at rows {0,32,64,96}."""
            srow = sc_rows[g]
            exp_f = exfp.tile([128, N], f32, tag="expf")
            nc.scalar.activation(exp_f[:], srow[:].rearrange("p a b -> p (a b)"),
                                 AF.Exp)
            se = smp.tile([128, 1], f32, tag="se")
            nc.vector.reduce_sum(se[:], exp_f[:], axis=mybir.AxisListType.X)
            rec = smp.tile([128, 1], f32, tag="rec")
            nc.vector.reciprocal(rec[:], se[:])
            wn_f = wnfp.tile([128, N], f32, tag="wnf")
            nc.vector.tensor_scalar_mul(wn_f[:], exp_f[:], rec[:])
            wn_b = smp.tile([128, N], bf16, tag="wnb")
            nc.vector.tensor_scalar_mul(wn_b[:], exp_f[:], rec[:])
            wn_fs[g] = wn_f
            wt = wtp.tile([128, NCH, 128], bf16, tag="wt")
            wn_bf[g] = wt
            # PE-mode transposes: the PE stream is idle between groups, and
            # this avoids the ~1.2us-per-DMA_TRANSPOSE ACT sequencer cost.
            for c in range(NCH):
                tp = tpp.tile([128, 128], bf16, tag="tp")
                nc.tensor.transpose(tp[:], wn_b[:, c * 128:(c + 1) * 128],
                                    id_sb[:])
                nc.vector.tensor_copy(wt[:, c, :], tp[:])

        def phase_c(b):
            """context = sum_n w[n] * img[b, n, :]."""
            g, k = divmod(b, 4)
            wt = wn_bf[g]
            cp = ctxps.tile([1, F], f32, tag="cp")
            for gg in range(NCH // CG):
                inat = inp.tile([128, CG, F], bf16, tag="in")
                # ACT ring: second HWDGE ring so both img streams feed the
                # 16 SDMA engines in parallel.
                nc.scalar.dma_start(inat[:], imgN[b, :, gg * CG:(gg + 1) * CG, :])
                for j in range(CG):
                    c = gg * CG + j
                    nc.tensor.matmul(
                        cp[:],
                        lhsT=wt[:, c, 32 * k:32 * k + 1],
                        rhs=inat[:, j, :],
                        start=(c == 0),
                        stop=(c == NCH - 1),
                    )
            csb = csp.tile([1, F], f32, tag="csb")
            nc.vector.tensor_copy(csb[:], cp[:])
            csbs[b] = csb

        # software pipeline: context(group 0) overlaps proj(group 1)
        for b in range(4):
            phase_a(b)
        softmax_group(0)
        for k in range(4):
            phase_a(4 + k)
            phase_c(k)
        softmax_group(1)
        for k in range(4):
            phase_c(4 + k)

        # all output DMAs at the end (ACT ring; SP ring stays pure input)
        for g in range(2):
            for kk in range(4):
                nc.scalar.dma_start(wts_out[g * 4 + kk],
                                    wn_fs[g][32 * kk:32 * kk + 1, :])
        for b in range(BPC):
            nc.scalar.dma_start(ctx_out[b:b + 1, :], csbs[b][:])

    nc.compile()
    _nc_cache["nc"] = nc
    return nc


def _in_maps(image_features, hidden_state, W_img, W_hid, W_score):
    img_bf = image_features.astype(_BF)                        # [64, 4096, 512]
    # context rhs: [B, p, n-chunk, f] with f contiguous per (p, chunk)
    imgN_q = np.ascontiguousarray(
        img_bf.reshape(B, NCH, 128, F).transpose(0, 2, 1, 3)
    )                                                          # [64, 128, 32, 512]
    # proj rhs: [B, p, f-chunk, n] with n contiguous per (p, chunk)
    imgT_bf = img_bf.transpose(0, 2, 1)                        # [64, 512, 4096] view
    imgT_q = np.ascontiguousarray(
        imgT_bf.reshape(B, 4, 128, N).transpose(0, 2, 1, 3)
    )                                                          # [64, 128, 4, 4096]
    wimgT = np.ascontiguousarray(W_img.T).astype(_BF)          # [512, 256]
    whidT = W_hid.T.astype(np.float32)                         # [512, 256]
    wsc = np.ascontiguousarray(W_score.reshape(1, D).T).astype(_BF)  # [256, 1]
    eye = np.eye(128, dtype=np.float32).astype(_BF)

    in_maps = []
    for c in range(NCORES):
        s = slice(c * BPC, (c + 1) * BPC)
        hwpack = np.concatenate(
            [whidT, hidden_state[s].T.astype(np.float32)], axis=1
        )  # [512, 264]
        in_maps.append({
            "imgT": imgT_q[s],
            "imgN": imgN_q[s],
            "hw": np.ascontiguousarray(hwpack),
            "wimgT": wimgT,
            "wscore": wsc,
            "ident": eye,
        })
    return in_maps


def kernel(image_features, hidden_state, W_img, W_hid, W_score):
    from concourse.bass_utils import run_bass_kernel_spmd

    nc = _build_nc()
    in_maps = _in_maps(image_features, hidden_state, W_img, W_hid, W_score)
    res = run_bass_kernel_spmd(nc, in_maps, list(range(NCORES))).results
    ctx = np.concatenate([r["ctx"] for r in res], axis=0)
    wts = np.concatenate([r["wts"] for r in res], axis=0)
    return (ctx, wts)


# revision 37
# speedup vs baseline: 1.0614x; 1.0614x over previous
"""Additive attention (Bahdanau) Trainium2 Bass kernel.

Sharding: data-parallel over batch B=64 -> 8 cores x 8 batches.
Per core, batches processed in 2 groups of 4 (score rows live at PSUM
partitions {0,32,64,96}):
  proj[d, n] = W_img @ img[b, n, :]           PE, [d,n] tiles, lhsT=W_imgT
  tanh tiles = tanh(proj + ph[d])             ACT, per-partition bias fused
  score row [1, 512] = W_score . tanh         PE, static [128,1] lhsT (row b%4*32)
  group softmax: exp/sum/normalize            ACT+DVE on [128, 4096] group tile
  w columns for context via DMA-xbar transpose of bf16 weights
  context = sum_n w[n] * img[b, n, :]         PE, lhsT = w column [128,1]

Host feeds partition-tiled layouts so every DMA has 4 KiB contiguous runs:
  imgT_q [8, 128, 4, 4096]  (p, f-chunk, n)  proj rhs tiles
  imgN_q [8, 128, 32, 512]  (p, n-chunk, f)  context rhs tiles

TRN2 instructions embed only ONE sync wait; Bacc's generate_event_semaphores
legalizes the rest, and ops are arranged to keep waits rare. Emission order
software-pipelines context(group 0) against proj(group 1).
"""

import sys
import numpy as np

for p in ("/opt/trn_rl_repo",):
    if p not in sys.path:
        sys.path.insert(0, p)

import ml_dtypes

B, N, F, H, D = 64, 4096, 512, 512, 256
NCORES = 8
BPC = B // NCORES  # batches per core
NT = 2048          # n-tile size for proj loads
NNT = N // NT      # 2 tiles
NSUB = NT // 512   # 4 x 512 matmul slices per tile
NCH = N // 128     # 32 n-chunks
CG = 4             # context chunks per load
_BF = ml_dtypes.bfloat16

_nc_cache = {}


def _build_nc():
    if "nc" in _nc_cache:
        return _nc_cache["nc"]
    from contextlib import ExitStack

    import concourse.bass as bass  # noqa: F401
    import concourse.tile as tile
    from concourse import bacc, mybir

    bf16 = mybir.dt.bfloat16
    f32 = mybir.dt.float32
    AF = mybir.ActivationFunctionType

    nc = bacc.Bacc("TRN2")

    imgT = nc.declare_dram_parameter("imgT", [BPC, 128, 4, N], bf16, isOutput=False)
    imgN = nc.declare_dram_parameter("imgN", [BPC, 128, NCH, F], bf16, isOutput=False)
    # hw = [W_hid.T | hidden.T] packed: one DMA -> one wait on the f32 matmul
    hw = nc.declare_dram_parameter("hw", [H, D + BPC], f32, isOutput=False)
    wimgT = nc.declare_dram_parameter("wimgT", [F, D], bf16, isOutput=False)
    wscore = nc.declare_dram_parameter("wscore", [D, 1], bf16, isOutput=False)
    ident = nc.declare_dram_parameter("ident", [128, 128], bf16, isOutput=False)
    ctx_out = nc.declare_dram_parameter("ctx", [BPC, F], f32, isOutput=True)
    wts_out = nc.declare_dram_parameter("wts", [BPC, N], f32, isOutput=True)

    with tile.TileContext(nc) as tc, ExitStack() as ctx:
        const = ctx.enter_context(tc.tile_pool(name="const", bufs=1))
        itp = ctx.enter_context(tc.tile_pool(name="imgTp", bufs=2))
        inp = ctx.enter_context(tc.tile_pool(name="imgNp", bufs=3))
        thp = ctx.enter_context(tc.tile_pool(name="tanh", bufs=4))
        scrp = ctx.enter_context(tc.tile_pool(name="scrp", bufs=1))
        wtp = ctx.enter_context(tc.tile_pool(name="wtp", bufs=1))
        exfp = ctx.enter_context(tc.tile_pool(name="exfp", bufs=1))
        wnfp = ctx.enter_context(tc.tile_pool(name="wnfp", bufs=2))
        smp = ctx.enter_context(tc.tile_pool(name="smp", bufs=2))
        csp = ctx.enter_context(tc.tile_pool(name="csb", bufs=BPC))
        pps = ctx.enter_context(tc.tile_pool(name="pp", bufs=4, space="PSUM"))
        scp = ctx.enter_context(tc.tile_pool(name="sc", bufs=1, space="PSUM"))
        tpp = ctx.enter_context(tc.tile_pool(name="tp", bufs=2, space="PSUM"))
        ctxps = ctx.enter_context(tc.tile_pool(name="cps", bufs=1, space="PSUM"))

        # ---- constants (consumers arranged for <=1 wait each) ----
        wimg_sb = const.tile([128, 4, D], bf16)
        nc.sync.dma_start(wimg_sb[:], wimgT.rearrange("(c p) d -> p c d", p=128))
        hw_sb = const.tile([128, 4, D + BPC], f32)
        nc.sync.dma_start(hw_sb[:], hw.rearrange("(c p) x -> p c x", p=128))
        wsc_dma = const.tile([128, 2, 1], bf16)
        nc.sync.dma_start(wsc_dma[:], wscore.rearrange("(c p) o -> p c o", p=128))
        wsc_sb = const.tile([128, 2, 1], bf16)
        nc.scalar.copy(wsc_sb[:], wsc_dma[:])
        id_sb = const.tile([128, 128], bf16)
        nc.sync.dma_start(id_sb[:], ident[:])

        # ---- proj_hidden in [d, b] layout (per-partition bias for tanh) ----
        ph_ps = tpp.tile([128, 2, BPC], f32, tag="tp")
        for dh in range(2):
            for hc in range(4):
                nc.tensor.matmul(
                    ph_ps[:, dh, :],
                    lhsT=hw_sb[:, hc, dh * 128:(dh + 1) * 128],
                    rhs=hw_sb[:, hc, D:D + BPC],
                    start=(hc == 0),
                    stop=(hc == 3),
                )
        ph_sb = const.tile([128, 2, BPC], f32)
        nc.scalar.copy(ph_sb[:], ph_ps[:])

        sc_rows = {}   # group -> [128, 8, 512] f32 score rows (4 live rows)
        wn_bf = {}     # group -> [128, NCH, 128] bf16 transposed weights
        wn_fs = {}     # group -> [128, N] f32 normalized weights (for output)
        csbs = {}      # batch -> [1, F] f32 context rows (for output)

        def phase_a(b):
            """proj + tanh + score rows for one batch."""
            g, k = divmod(b, 4)
            if k == 0:
                srow_new = scrp.tile([128, NNT * NSUB, 512], f32, tag="scr")
                sc_rows[g] = srow_new
            srow = sc_rows[g]
            for nt in range(NNT):
                it = itp.tile([128, 4, NT], bf16, tag="it")
                nc.sync.dma_start(it[:], imgT[b, :, :, nt * NT:(nt + 1) * NT])
                for sub in range(NSUB):
                    sl = slice(sub * 512, (sub + 1) * 512)
                    ths = []
                    for dh in range(2):
                        pp = pps.tile([128, 512], f32, tag="pp")
                        for fc in range(4):
                            nc.tensor.matmul(
                                pp[:],
                                lhsT=wimg_sb[:, fc, dh * 128:(dh + 1) * 128],
                                rhs=it[:, fc, sl],
                                start=(fc == 0),
                                stop=(fc == 3),
                            )
                        th = thp.tile([128, 512], bf16, tag="th")
                        nc.scalar.activation(
                            th[:], pp[:], AF.Tanh,
                            bias=ph_sb[:, dh, b:b + 1], scale=1.0,
                        )
                        ths.append(th)
                    sc = scp.tile([128, 512], f32, tag="sc")
                    r = 32 * k
                    for dh in range(2):
                        nc.tensor.matmul(
                            sc[r:r + 1, :],
                            lhsT=wsc_sb[:, dh, :],
                            rhs=ths[dh][:],
                            start=(dh == 0),
                            stop=(dh == 1),
                            tile_position=(0, r),
                        )
                    nc.vector.tensor_copy(
                        srow[r:r + 1, nt * NSUB + sub, :], sc[r:r + 1, :]
                    )

        def softmax_group(g):
            """exp/sum/normalize/transpose for 4 batches at rows {0,32,64,96}."""
            srow = sc_rows[g]
            exp_f = exfp.tile([128, N], f32, tag="expf")
            nc.scalar.activation(exp_f[:], srow[:].rearrange("p a b -> p (a b)"),
                                 AF.Exp)
            se = smp.tile([128, 1], f32, tag="se")
            nc.vector.reduce_sum(se[:], exp_f[:], axis=mybir.AxisListType.X)
            rec = smp.tile([128, 1], f32, tag="rec")
            nc.vector.reciprocal(rec[:], se[:])
            wn_f = wnfp.tile([128, N], f32, tag="wnf")
            nc.vector.tensor_scalar_mul(wn_f[:], exp_f[:], rec[:])
            wn_b = smp.tile([128, N], bf16, tag="wnb")
            nc.vector.tensor_scalar_mul(wn_b[:], exp_f[:], rec[:])
            wn_fs[g] = wn_f
            wt = wtp.tile([128, NCH, 128], bf16, tag="wt")
            wn_bf[g] = wt
            # PE-mode transposes: the PE stream is idle between groups, and
            # this avoids the ~1.2us-per-DMA_TRANSPOSE ACT sequencer cost.
            for c in range(NCH):
                tp = tpp.tile([128, 128], bf16, tag="tp")
                nc.tensor.transpose(tp[:], wn_b[:, c * 128:(c + 1) * 128],
                                    id_sb[:])
                nc.vector.tensor_copy(wt[:, c, :], tp[:])

        def phase_c(b):
            """context = sum_n w[n] * img[b, n, :]."""
            g, k = divmod(b, 4)
            wt = wn_bf[g]
            cp = ctxps.tile([1, F], f32, tag="cp")
            for gg in range(NCH // CG):
                inat = inp.tile([128, CG, F], bf16, tag="in")
                nc.sync.dma_start(inat[:], imgN[b, :, gg * CG:(gg + 1) * CG, :])
                for j in range(CG):
                    c = gg * CG + j
                    nc.tensor.matmul(
                        cp[:],
                        lhsT=wt[:, c, 32 * k:32 * k + 1],
                        rhs=inat[:, j, :],
                        start=(c == 0),
                        stop=(c == NCH - 1),
                    )
            csb = csp.tile([1, F], f32, tag="csb")
            nc.vector.tensor_copy(csb[:], cp[:])
            csbs[b] = csb

        # software pipeline: context(group 0) overlaps proj(group 1)
        for b in range(4):
            phase_a(b)
        softmax_group(0)
        for k in range(4):
            phase_a(4 + k)
            phase_c(k)
        softmax_group(1)
        for k in range(4):
            phase_c(4 + k)

        # all output DMAs at the end (ACT ring; SP ring stays pure input)
        for g in range(2):
            for kk in range(4):
                nc.scalar.dma_start(wts_out[g * 4 + kk],
                                    wn_fs[g][32 * kk:32 * kk + 1, :])
        for b in range(BPC):
            nc.scalar.dma_start(ctx_out[b:b + 1, :], csbs[b][:])

    nc.compile()
    _nc_cache["nc"] = nc
    return nc


def _in_maps(image_features, hidden_state, W_img, W_hid, W_score):
    img_bf = image_features.astype(_BF)                        # [64, 4096, 512]
    # context rhs: [B, p, n-chunk, f] with f contiguous per (p, chunk)
    imgN_q = np.ascontiguousarray(
        img_bf.reshape(B, NCH, 128, F).transpose(0, 2, 1, 3)
    )                                                          # [64, 128, 32, 512]
    # proj rhs: [B, p, f-chunk, n] with n contiguous per (p, chunk)
    imgT_bf = img_bf.transpose(0, 2, 1)                        # [64, 512, 4096] view
    imgT_q = np.ascontiguousarray(
        imgT_bf.reshape(B, 4, 128, N).transpose(0, 2, 1, 3)
    )                                                          # [64, 128, 4, 4096]
    wimgT = np.ascontiguousarray(W_img.T).astype(_BF)          # [512, 256]
    whidT = W_hid.T.astype(np.float32)                         # [512, 256]
    wsc = np.ascontiguousarray(W_score.reshape(1, D).T).astype(_BF)  # [256, 1]
    eye = np.eye(128, dtype=np.float32).astype(_BF)

    in_maps = []
    for c in range(NCORES):
        s = slice(c * BPC, (c + 1) * BPC)
        hwpack = np.concatenate(
            [whidT, hidden_state[s].T.astype(np.float32)], axis=1
        )  # [512, 264]
        in_maps.append({
            "imgT": imgT_q[s],
            "imgN": imgN_q[s],
            "hw": np.ascontiguousarray(hwpack),
            "wimgT": wimgT,
            "wscore": wsc,
            "ident": eye,
        })
    return in_maps


def kernel(image_features, hidden_state, W_img, W_hid, W_score):
    from concourse.bass_utils import run_bass_kernel_spmd

    nc = _build_nc()
    in_maps = _in_maps(image_features, hidden_state, W_img, W_hid, W_score)
    res = run_bass_kernel_spmd(nc, in_maps, list(range(NCORES))).results
    ctx = np.concatenate([r["ctx"] for r in res], axis=0)
    wts = np.concatenate([r["wts"] for r in res], axis=0)
    return (ctx, wts)


# revision 42
# speedup vs baseline: 1.1429x; 1.0768x over previous
"""Additive attention (Bahdanau) Trainium2 Bass kernel.

Sharding: data-parallel over batch B=64 -> 8 cores x 8 batches.
Per core, batches processed in 2 groups of 4 (score rows live at PSUM
partitions {0,32,64,96}):
  proj[d, n] = W_img @ img[b, n, :]           PE, [d,n] tiles, lhsT=W_imgT
  tanh tiles = tanh(proj + ph[d])             ACT, per-partition bias fused
  score row [1, 512] = W_score . tanh         PE, static [128,1] lhsT (row b%4*32)
  group softmax: exp/sum/normalize            ACT+DVE on [128, 4096] group tile
  w columns for context via DMA-xbar transpose of bf16 weights
  context = sum_n w[n] * img[b, n, :]         PE, lhsT = w column [128,1]

Host feeds partition-tiled layouts so every DMA has 4 KiB contiguous runs:
  imgT_q [8, 128, 4, 4096]  (p, f-chunk, n)  proj rhs tiles
  imgN_q [8, 128, 32, 512]  (p, n-chunk, f)  context rhs tiles

TRN2 instructions embed only ONE sync wait; Bacc's generate_event_semaphores
legalizes the rest, and ops are arranged to keep waits rare. Emission order
software-pipelines context(group 0) against proj(group 1).
"""

import sys
import numpy as np

for p in ("/opt/trn_rl_repo",):
    if p not in sys.path:
        sys.path.insert(0, p)

import ml_dtypes

B, N, F, H, D = 64, 4096, 512, 512, 256
NCORES = 8
BPC = B // NCORES  # batches per core
NT = 2048          # n-tile size for proj loads
NNT = N // NT      # 2 tiles
NSUB = NT // 512   # 4 x 512 matmul slices per tile
NCH = N // 128     # 32 n-chunks
CG = 4             # context chunks per load
_BF = ml_dtypes.bfloat16

_nc_cache = {}


def _build_nc():
    if "nc" in _nc_cache:
        return _nc_cache["nc"]
    from contextlib import ExitStack

    import concourse.bass as bass  # noqa: F401
    import concourse.tile as tile
    from concourse import bacc, mybir

    bf16 = mybir.dt.bfloat16
    f32 = mybir.dt.float32
    AF = mybir.ActivationFunctionType

    nc = bacc.Bacc("TRN2")

    imgT = nc.declare_dram_parameter("imgT", [BPC, 128, 4, N], bf16, isOutput=False)
    imgN = nc.declare_dram_parameter("imgN", [BPC, 128, NCH, F], bf16, isOutput=False)
    # hw = [W_hid.T | hidden.T] packed: one DMA -> one wait on the f32 matmul
    hw = nc.declare_dram_parameter("hw", [H, D + BPC], f32, isOutput=False)
    wimgT = nc.declare_dram_parameter("wimgT", [F, D], bf16, isOutput=False)
    wscore = nc.declare_dram_parameter("wscore", [D, 1], bf16, isOutput=False)
    ident = nc.declare_dram_parameter("ident", [128, 128], bf16, isOutput=False)
    ctx_out = nc.declare_dram_parameter("ctx", [BPC, F], f32, isOutput=True)
    wts_out = nc.declare_dram_parameter("wts", [BPC, N], f32, isOutput=True)

    with tile.TileContext(nc) as tc, ExitStack() as ctx:
        const = ctx.enter_context(tc.tile_pool(name="const", bufs=1))
        itp = ctx.enter_context(tc.tile_pool(name="imgTp", bufs=2))
        inp = ctx.enter_context(tc.tile_pool(name="imgNp", bufs=3))
        thp = ctx.enter_context(tc.tile_pool(name="tanh", bufs=6))
        scrp = ctx.enter_context(tc.tile_pool(name="scrp", bufs=1))
        wtp = ctx.enter_context(tc.tile_pool(name="wtp", bufs=1))
        exfp = ctx.enter_context(tc.tile_pool(name="exfp", bufs=1))
        wnfp = ctx.enter_context(tc.tile_pool(name="wnfp", bufs=2))
        smp = ctx.enter_context(tc.tile_pool(name="smp", bufs=2))
        csp = ctx.enter_context(tc.tile_pool(name="csb", bufs=BPC))
        pps = ctx.enter_context(tc.tile_pool(name="pp", bufs=3, space="PSUM"))
        scp = ctx.enter_context(tc.tile_pool(name="sc", bufs=2, space="PSUM"))
        tpp = ctx.enter_context(tc.tile_pool(name="tp", bufs=2, space="PSUM"))
        ctxps = ctx.enter_context(tc.tile_pool(name="cps", bufs=1, space="PSUM"))

        # ---- constants (consumers arranged for <=1 wait each) ----
        wimg_sb = const.tile([128, 4, D], bf16)
        nc.sync.dma_start(wimg_sb[:], wimgT.rearrange("(c p) d -> p c d", p=128))
        hw_sb = const.tile([128, 4, D + BPC], f32)
        nc.sync.dma_start(hw_sb[:], hw.rearrange("(c p) x -> p c x", p=128))
        wsc_dma = const.tile([128, 2, 1], bf16)
        nc.sync.dma_start(wsc_dma[:], wscore.rearrange("(c p) o -> p c o", p=128))
        wsc_sb = const.tile([128, 2, 1], bf16)
        nc.scalar.copy(wsc_sb[:], wsc_dma[:])
        id_sb = const.tile([128, 128], bf16)
        nc.sync.dma_start(id_sb[:], ident[:])

        # ---- proj_hidden in [d, b] layout (per-partition bias for tanh) ----
        ph_ps = tpp.tile([128, 2, BPC], f32, tag="tp")
        for dh in range(2):
            for hc in range(4):
                nc.tensor.matmul(
                    ph_ps[:, dh, :],
                    lhsT=hw_sb[:, hc, dh * 128:(dh + 1) * 128],
                    rhs=hw_sb[:, hc, D:D + BPC],
                    start=(hc == 0),
                    stop=(hc == 3),
                )
        ph_sb = const.tile([128, 2, BPC], f32)
        nc.scalar.copy(ph_sb[:], ph_ps[:])

        sc_rows = {}   # group -> [128, 8, 512] f32 score rows (4 live rows)
        wn_bf = {}     # group -> [128, NCH, 128] bf16 transposed weights
        wn_fs = {}     # group -> [128, N] f32 normalized weights (for output)
        csbs = {}      # batch -> [1, F] f32 context rows (for output)

        def phase_a(b):
            """proj + tanh + score rows for one batch."""
            g, k = divmod(b, 4)
            if k == 0:
                srow_new = scrp.tile([128, NNT * NSUB, 512], f32, tag="scr")
                sc_rows[g] = srow_new
            srow = sc_rows[g]
            r = 32 * k
            pend = None  # (ths, chunk-col) for deferred scores matmuls

            def flush_scores():
                nonlocal pend
                if pend is None:
                    return
                ths, cc = pend
                sc = scp.tile([128, 512], f32, tag="sc")
                for dh in range(2):
                    nc.tensor.matmul(
                        sc[r:r + 1, :],
                        lhsT=wsc_sb[:, dh, :],
                        rhs=ths[dh][:],
                        start=(dh == 0),
                        stop=(dh == 1),
                        tile_position=(0, r),
                    )
                nc.vector.tensor_copy(srow[r:r + 1, cc, :], sc[r:r + 1, :])
                pend = None

            for nt in range(NNT):
                it = itp.tile([128, 4, NT], bf16, tag="it")
                nc.sync.dma_start(it[:], imgT[b, :, :, nt * NT:(nt + 1) * NT])
                for sub in range(NSUB):
                    sl = slice(sub * 512, (sub + 1) * 512)
                    ths = []
                    for dh in range(2):
                        pp = pps.tile([128, 512], f32, tag="pp")
                        for fc in range(4):
                            nc.tensor.matmul(
                                pp[:],
                                lhsT=wimg_sb[:, fc, dh * 128:(dh + 1) * 128],
                                rhs=it[:, fc, sl],
                                start=(fc == 0),
                                stop=(fc == 3),
                            )
                        th = thp.tile([128, 512], bf16, tag="th")
                        nc.scalar.activation(
                            th[:], pp[:], AF.Tanh,
                            bias=ph_sb[:, dh, b:b + 1], scale=1.0,
                        )
                        ths.append(th)
                    flush_scores()
                    pend = (ths, nt * NSUB + sub)
            flush_scores()

        def softmax_group(g):
            """exp/sum/normalize/transpose for 4 batches at rows {0,32,64,96}."""
            srow = sc_rows[g]
            exp_f = exfp.tile([128, N], f32, tag="expf")
            nc.scalar.activation(exp_f[:], srow[:].rearrange("p a b -> p (a b)"),
                                 AF.Exp)
            se = smp.tile([128, 1], f32, tag="se")
            nc.vector.reduce_sum(se[:], exp_f[:], axis=mybir.AxisListType.X)
            rec = smp.tile([128, 1], f32, tag="rec")
            nc.vector.reciprocal(rec[:], se[:])
            wn_f = wnfp.tile([128, N], f32, tag="wnf")
            nc.vector.tensor_scalar_mul(wn_f[:], exp_f[:], rec[:])
            wn_b = smp.tile([128, N], bf16, tag="wnb")
            nc.vector.tensor_scalar_mul(wn_b[:], exp_f[:], rec[:])
            wn_fs[g] = wn_f
            wt = wtp.tile([128, NCH, 128], bf16, tag="wt")
            wn_bf[g] = wt
            # PE-mode transposes: the PE stream is idle between groups, and
            # this avoids the ~1.2us-per-DMA_TRANSPOSE ACT sequencer cost.
            for c in range(NCH):
                tp = tpp.tile([128, 128], bf16, tag="tp")
                nc.tensor.transpose(tp[:], wn_b[:, c * 128:(c + 1) * 128],
                                    id_sb[:])
                nc.vector.tensor_copy(wt[:, c, :], tp[:])

        def phase_c(b):
            """context = sum_n w[n] * img[b, n, :]."""
            g, k = divmod(b, 4)
            wt = wn_bf[g]
            cp = ctxps.tile([1, F], f32, tag="cp")
            for gg in range(NCH // CG):
                inat = inp.tile([128, CG, F], bf16, tag="in")
                nc.sync.dma_start(inat[:], imgN[b, :, gg * CG:(gg + 1) * CG, :])
                for j in range(CG):
                    c = gg * CG + j
                    nc.tensor.matmul(
                        cp[:],
                        lhsT=wt[:, c, 32 * k:32 * k + 1],
                        rhs=inat[:, j, :],
                        start=(c == 0),
                        stop=(c == NCH - 1),
                    )
            csb = csp.tile([1, F], f32, tag="csb")
            nc.vector.tensor_copy(csb[:], cp[:])
            csbs[b] = csb

        # software pipeline: context(group 0) overlaps proj(group 1)
        for b in range(4):
            phase_a(b)
        softmax_group(0)
        for k in range(4):
            phase_a(4 + k)
            phase_c(k)
        softmax_group(1)
        for k in range(4):
            phase_c(4 + k)

        # all output DMAs at the end (ACT ring; SP ring stays pure input)
        for g in range(2):
            for kk in range(4):
                nc.scalar.dma_start(wts_out[g * 4 + kk],
                                    wn_fs[g][32 * kk:32 * kk + 1, :])
        for b in range(BPC):
            nc.scalar.dma_start(ctx_out[b:b + 1, :], csbs[b][:])

    nc.compile()
    _nc_cache["nc"] = nc
    return nc


def _in_maps(image_features, hidden_state, W_img, W_hid, W_score):
    img_bf = image_features.astype(_BF)                        # [64, 4096, 512]
    # context rhs: [B, p, n-chunk, f] with f contiguous per (p, chunk)
    imgN_q = np.ascontiguousarray(
        img_bf.reshape(B, NCH, 128, F).transpose(0, 2, 1, 3)
    )                                                          # [64, 128, 32, 512]
    # proj rhs: [B, p, f-chunk, n] with n contiguous per (p, chunk)
    imgT_bf = img_bf.transpose(0, 2, 1)                        # [64, 512, 4096] view
    imgT_q = np.ascontiguousarray(
        imgT_bf.reshape(B, 4, 128, N).transpose(0, 2, 1, 3)
    )                                                          # [64, 128, 4, 4096]
    wimgT = np.ascontiguousarray(W_img.T).astype(_BF)          # [512, 256]
    whidT = W_hid.T.astype(np.float32)                         # [512, 256]
    wsc = np.ascontiguousarray(W_score.reshape(1, D).T).astype(_BF)  # [256, 1]
    eye = np.eye(128, dtype=np.float32).astype(_BF)

    in_maps = []
    for c in range(NCORES):
        s = slice(c * BPC, (c + 1) * BPC)
        hwpack = np.concatenate(
            [whidT, hidden_state[s].T.astype(np.float32)], axis=1
        )  # [512, 264]
        in_maps.append({
            "imgT": imgT_q[s],
            "imgN": imgN_q[s],
            "hw": np.ascontiguousarray(hwpack),
            "wimgT": wimgT,
            "wscore": wsc,
            "ident": eye,
        })
    return in_maps


def kernel(image_features, hidden_state, W_img, W_hid, W_score):
    from concourse.bass_utils import run_bass_kernel_spmd

    nc = _build_nc()
    in_maps = _in_maps(image_features, hidden_state, W_img, W_hid, W_score)
    res = run_bass_kernel_spmd(nc, in_maps, list(range(NCORES))).results
    ctx = np.concatenate([r["ctx"] for r in res], axis=0)
    wts = np.concatenate([r["wts"] for r in res], axis=0)
    return (ctx, wts)


# revision 47
# speedup vs baseline: 1.1963x; 1.0467x over previous
"""Additive attention (Bahdanau) Trainium2 Bass kernel.

Sharding: data-parallel over batch B=64 -> 8 cores x 8 batches.
Per core, batches processed in 2 groups of 4 (score rows live at PSUM
partitions {0,32,64,96}):
  proj[d, n] = W_img @ img[b, n, :]           PE, [d,n] tiles, lhsT=W_imgT
  tanh tiles = tanh(proj + ph[d])             ACT, per-partition bias fused
  score row [1, 512] = W_score . tanh         PE, static [128,1] lhsT (row b%4*32)
  group softmax: exp/sum/normalize            ACT+DVE on [128, 4096] group tile
  w columns for context via DMA-xbar transpose of bf16 weights
  context = sum_n w[n] * img[b, n, :]         PE, lhsT = w column [128,1]

Host feeds partition-tiled layouts so every DMA has 4 KiB contiguous runs:
  imgT_q [8, 128, 4, 4096]  (p, f-chunk, n)  proj rhs tiles
  imgN_q [8, 128, 32, 512]  (p, n-chunk, f)  context rhs tiles

TRN2 instructions embed only ONE sync wait; Bacc's generate_event_semaphores
legalizes the rest, and ops are arranged to keep waits rare. Emission order
software-pipelines context(group 0) against proj(group 1).
"""

import sys
import numpy as np

for p in ("/opt/trn_rl_repo",):
    if p not in sys.path:
        sys.path.insert(0, p)

import ml_dtypes

B, N, F, H, D = 64, 4096, 512, 512, 256
NCORES = 8
BPC = B // NCORES  # batches per core
NT = 2048          # n-tile size for proj loads
NNT = N // NT      # 2 tiles
NSUB = NT // 512   # 4 x 512 matmul slices per tile
NCH = N // 128     # 32 n-chunks
CG = 4             # context chunks per load
_BF = ml_dtypes.bfloat16

_nc_cache = {}


def _build_nc():
    if "nc" in _nc_cache:
        return _nc_cache["nc"]
    from contextlib import ExitStack

    import concourse.bass as bass  # noqa: F401
    import concourse.tile as tile
    from concourse import bacc, mybir

    bf16 = mybir.dt.bfloat16
    f32 = mybir.dt.float32
    AF = mybir.ActivationFunctionType

    nc = bacc.Bacc("TRN2")

    imgT = nc.declare_dram_parameter("imgT", [BPC, 128, 4, N], bf16, isOutput=False)
    imgN = nc.declare_dram_parameter("imgN", [BPC, 128, NCH, F], bf16, isOutput=False)
    # hw = [W_hid.T | hidden.T] packed: one DMA -> one wait on the f32 matmul
    hw = nc.declare_dram_parameter("hw", [H, D + BPC], f32, isOutput=False)
    wimgT = nc.declare_dram_parameter("wimgT", [F, D], bf16, isOutput=False)
    wscore = nc.declare_dram_parameter("wscore", [D, 1], bf16, isOutput=False)
    ident = nc.declare_dram_parameter("ident", [128, 128], bf16, isOutput=False)
    ctx_out = nc.declare_dram_parameter("ctx", [BPC, F], f32, isOutput=True)
    wts_out = nc.declare_dram_parameter("wts", [BPC, N], f32, isOutput=True)

    with tile.TileContext(nc) as tc, ExitStack() as ctx:
        const = ctx.enter_context(tc.tile_pool(name="const", bufs=1))
        itp = ctx.enter_context(tc.tile_pool(name="imgTp", bufs=2))
        inp = ctx.enter_context(tc.tile_pool(name="imgNp", bufs=4))
        thp = ctx.enter_context(tc.tile_pool(name="tanh", bufs=6))
        scrp = ctx.enter_context(tc.tile_pool(name="scrp", bufs=1))
        wtp = ctx.enter_context(tc.tile_pool(name="wtp", bufs=1))
        exfp = ctx.enter_context(tc.tile_pool(name="exfp", bufs=1))
        wnfp = ctx.enter_context(tc.tile_pool(name="wnfp", bufs=2))
        smp = ctx.enter_context(tc.tile_pool(name="smp", bufs=2))
        csp = ctx.enter_context(tc.tile_pool(name="csb", bufs=BPC))
        pps = ctx.enter_context(tc.tile_pool(name="pp", bufs=3, space="PSUM"))
        scp = ctx.enter_context(tc.tile_pool(name="sc", bufs=2, space="PSUM"))
        tpp = ctx.enter_context(tc.tile_pool(name="tp", bufs=2, space="PSUM"))
        ctxps = ctx.enter_context(tc.tile_pool(name="cps", bufs=1, space="PSUM"))

        # ---- constants (consumers arranged for <=1 wait each) ----
        wimg_sb = const.tile([128, 4, D], bf16)
        nc.sync.dma_start(wimg_sb[:], wimgT.rearrange("(c p) d -> p c d", p=128))
        hw_sb = const.tile([128, 4, D + BPC], f32)
        nc.sync.dma_start(hw_sb[:], hw.rearrange("(c p) x -> p c x", p=128))
        wsc_dma = const.tile([128, 2, 1], bf16)
        nc.sync.dma_start(wsc_dma[:], wscore.rearrange("(c p) o -> p c o", p=128))
        wsc_sb = const.tile([128, 2, 1], bf16)
        nc.scalar.copy(wsc_sb[:], wsc_dma[:])
        id_sb = const.tile([128, 128], bf16)
        nc.sync.dma_start(id_sb[:], ident[:])

        # ---- proj_hidden in [d, b] layout (per-partition bias for tanh) ----
        ph_ps = tpp.tile([128, 2, BPC], f32, tag="tp")
        for dh in range(2):
            for hc in range(4):
                nc.tensor.matmul(
                    ph_ps[:, dh, :],
                    lhsT=hw_sb[:, hc, dh * 128:(dh + 1) * 128],
                    rhs=hw_sb[:, hc, D:D + BPC],
                    start=(hc == 0),
                    stop=(hc == 3),
                )
        ph_sb = const.tile([128, 2, BPC], f32)
        nc.scalar.copy(ph_sb[:], ph_ps[:])

        sc_rows = {}   # group -> [128, 8, 512] f32 score rows (4 live rows)
        wn_bf = {}     # group -> [128, NCH, 128] bf16 transposed weights
        wn_fs = {}     # group -> [128, N] f32 normalized weights (for output)
        csbs = {}      # batch -> [1, F] f32 context rows (for output)

        def phase_a_steps(b):
            """proj + tanh + score rows for one batch; yields after each sub."""
            g, k = divmod(b, 4)
            if k == 0:
                srow_new = scrp.tile([128, NNT * NSUB, 512], f32, tag="scr")
                sc_rows[g] = srow_new
            srow = sc_rows[g]
            r = 32 * k
            pend = None  # (ths, chunk-col) for deferred scores matmuls

            def flush_scores():
                nonlocal pend
                if pend is None:
                    return
                ths, cc = pend
                sc = scp.tile([128, 512], f32, tag="sc")
                for dh in range(2):
                    nc.tensor.matmul(
                        sc[r:r + 1, :],
                        lhsT=wsc_sb[:, dh, :],
                        rhs=ths[dh][:],
                        start=(dh == 0),
                        stop=(dh == 1),
                        tile_position=(0, r),
                    )
                nc.vector.tensor_copy(srow[r:r + 1, cc, :], sc[r:r + 1, :])
                pend = None

            for nt in range(NNT):
                it = itp.tile([128, 4, NT], bf16, tag="it")
                nc.sync.dma_start(it[:], imgT[b, :, :, nt * NT:(nt + 1) * NT])
                for sub in range(NSUB):
                    sl = slice(sub * 512, (sub + 1) * 512)
                    ths = []
                    for dh in range(2):
                        pp = pps.tile([128, 512], f32, tag="pp")
                        for fc in range(4):
                            nc.tensor.matmul(
                                pp[:],
                                lhsT=wimg_sb[:, fc, dh * 128:(dh + 1) * 128],
                                rhs=it[:, fc, sl],
                                start=(fc == 0),
                                stop=(fc == 3),
                            )
                        th = thp.tile([128, 512], bf16, tag="th")
                        nc.scalar.activation(
                            th[:], pp[:], AF.Tanh,
                            bias=ph_sb[:, dh, b:b + 1], scale=1.0,
                        )
                        ths.append(th)
                    flush_scores()
                    pend = (ths, nt * NSUB + sub)
                    yield
            flush_scores()

        def softmax_group(g):
            """exp/sum/normalize/transpose for 4 batches at rows {0,32,64,96}."""
            srow = sc_rows[g]
            exp_f = exfp.tile([128, N], f32, tag="expf")
            nc.scalar.activation(exp_f[:], srow[:].rearrange("p a b -> p (a b)"),
                                 AF.Exp)
            se = smp.tile([128, 1], f32, tag="se")
            nc.vector.reduce_sum(se[:], exp_f[:], axis=mybir.AxisListType.X)
            rec = smp.tile([128, 1], f32, tag="rec")
            nc.vector.reciprocal(rec[:], se[:])
            wn_f = wnfp.tile([128, N], f32, tag="wnf")
            nc.vector.tensor_scalar_mul(wn_f[:], exp_f[:], rec[:])
            wn_b = smp.tile([128, N], bf16, tag="wnb")
            nc.vector.tensor_scalar_mul(wn_b[:], exp_f[:], rec[:])
            wn_fs[g] = wn_f
            wt = wtp.tile([128, NCH, 128], bf16, tag="wt")
            wn_bf[g] = wt
            # PE-mode transposes: the PE stream is idle between groups, and
            # this avoids the ~1.2us-per-DMA_TRANSPOSE ACT sequencer cost.
            for c in range(NCH):
                tp = tpp.tile([128, 128], bf16, tag="tp")
                nc.tensor.transpose(tp[:], wn_b[:, c * 128:(c + 1) * 128],
                                    id_sb[:])
                nc.vector.tensor_copy(wt[:, c, :], tp[:])

        def phase_c_steps(b):
            """context = sum_n w[n] * img[b, n, :], as 8 resumable steps."""
            g, k = divmod(b, 4)
            wt = wn_bf[g]
            cp = ctxps.tile([1, F], f32, tag="cp")
            for gg in range(NCH // CG):
                inat = inp.tile([128, CG, F], bf16, tag="in")
                nc.sync.dma_start(inat[:], imgN[b, :, gg * CG:(gg + 1) * CG, :])
                for j in range(CG):
                    c = gg * CG + j
                    nc.tensor.matmul(
                        cp[:],
                        lhsT=wt[:, c, 32 * k:32 * k + 1],
                        rhs=inat[:, j, :],
                        start=(c == 0),
                        stop=(c == NCH - 1),
                    )
                yield
            csb = csp.tile([1, F], f32, tag="csb")
            nc.vector.tensor_copy(csb[:], cp[:])
            csbs[b] = csb
            while True:
                yield

        # software pipeline: context(group 0) overlaps proj(group 1)
        for b in range(4):
            for _ in phase_a_steps(b):
                pass
        softmax_group(0)
        for k in range(4):
            agen = phase_a_steps(4 + k)
            cgen = phase_c_steps(k)
            alive = True
            while alive:
                alive = next(agen, "end") != "end"
                next(cgen)
        softmax_group(1)
        for k in range(4):
            cgen = phase_c_steps(4 + k)
            for _ in range(NCH // CG + 1):
                next(cgen)

        # all output DMAs at the end (ACT ring; SP ring stays pure input)
        for g in range(2):
            for kk in range(4):
                nc.scalar.dma_start(wts_out[g * 4 + kk],
                                    wn_fs[g][32 * kk:32 * kk + 1, :])
        for b in range(BPC):
            nc.scalar.dma_start(ctx_out[b:b + 1, :], csbs[b][:])

    nc.compile()
    _nc_cache["nc"] = nc
    return nc


def _in_maps(image_features, hidden_state, W_img, W_hid, W_score):
    img_bf = image_features.astype(_BF)                        # [64, 4096, 512]
    # context rhs: [B, p, n-chunk, f] with f contiguous per (p, chunk)
    imgN_q = np.ascontiguousarray(
        img_bf.reshape(B, NCH, 128, F).transpose(0, 2, 1, 3)
    )                                                          # [64, 128, 32, 512]
    # proj rhs: [B, p, f-chunk, n] with n contiguous per (p, chunk)
    imgT_bf = img_bf.transpose(0, 2, 1)                        # [64, 512, 4096] view
    imgT_q = np.ascontiguousarray(
        imgT_bf.reshape(B, 4, 128, N).transpose(0, 2, 1, 3)
    )                                                          # [64, 128, 4, 4096]
    wimgT = np.ascontiguousarray(W_img.T).astype(_BF)          # [512, 256]
    whidT = W_hid.T.astype(np.float32)                         # [512, 256]
    wsc = np.ascontiguousarray(W_score.reshape(1, D).T).astype(_BF)  # [256, 1]
    eye = np.eye(128, dtype=np.float32).astype(_BF)

    in_maps = []
    for c in range(NCORES):
        s = slice(c * BPC, (c + 1) * BPC)
        hwpack = np.concatenate(
            [whidT, hidden_state[s].T.astype(np.float32)], axis=1
        )  # [512, 264]
        in_maps.append({
            "imgT": imgT_q[s],
            "imgN": imgN_q[s],
            "hw": np.ascontiguousarray(hwpack),
            "wimgT": wimgT,
            "wscore": wsc,
            "ident": eye,
        })
    return in_maps


def kernel(image_features, hidden_state, W_img, W_hid, W_score):
    from concourse.bass_utils import run_bass_kernel_spmd

    nc = _build_nc()
    in_maps = _in_maps(image_features, hidden_state, W_img, W_hid, W_score)
    res = run_bass_kernel_spmd(nc, in_maps, list(range(NCORES))).results
    ctx = np.concatenate([r["ctx"] for r in res], axis=0)
    wts = np.concatenate([r["wts"] for r in res], axis=0)
    return (ctx, wts)


# revision 53
# speedup vs baseline: 1.2406x; 1.0370x over previous
"""Additive attention (Bahdanau) Trainium2 Bass kernel.

Sharding: data-parallel over batch B=64 -> 8 cores x 8 batches.
Per core, batches processed in 2 groups of 4 (score rows live at PSUM
partitions {0,32,64,96}):
  proj[d, n] = W_img @ img[b, n, :]           PE, [d,n] tiles, lhsT=W_imgT
  tanh tiles = tanh(proj + ph[d])             ACT, per-partition bias fused
  score row [1, 512] = W_score . tanh         PE, static [128,1] lhsT (row b%4*32)
  group softmax: exp/sum/normalize            ACT+DVE on [128, 4096] group tile
  w columns for context via DMA-xbar transpose of bf16 weights
  context = sum_n w[n] * img[b, n, :]         PE, lhsT = w column [128,1]

Host feeds partition-tiled layouts so every DMA has 4 KiB contiguous runs:
  imgT_q [8, 128, 4, 4096]  (p, f-chunk, n)  proj rhs tiles
  imgN_q [8, 128, 32, 512]  (p, n-chunk, f)  context rhs tiles

TRN2 instructions embed only ONE sync wait; Bacc's generate_event_semaphores
legalizes the rest, and ops are arranged to keep waits rare. Emission order
software-pipelines context(group 0) against proj(group 1).
"""

import sys
import numpy as np

for p in ("/opt/trn_rl_repo",):
    if p not in sys.path:
        sys.path.insert(0, p)

import ml_dtypes

B, N, F, H, D = 64, 4096, 512, 512, 256
NCORES = 8
BPC = B // NCORES  # batches per core
NT = 2048          # n-tile size for proj loads
NNT = N // NT      # 2 tiles
NSUB = NT // 512   # 4 x 512 matmul slices per tile
NCH = N // 128     # 32 n-chunks
CG = 4             # context chunks per load
_BF = ml_dtypes.bfloat16

_nc_cache = {}


def _build_nc():
    if "nc" in _nc_cache:
        return _nc_cache["nc"]
    from contextlib import ExitStack

    import concourse.bass as bass  # noqa: F401
    import concourse.tile as tile
    from concourse import bacc, mybir

    bf16 = mybir.dt.bfloat16
    f32 = mybir.dt.float32
    AF = mybir.ActivationFunctionType

    nc = bacc.Bacc("TRN2")

    imgT = nc.declare_dram_parameter("imgT", [BPC, 128, 4, N], bf16, isOutput=False)
    imgN = nc.declare_dram_parameter("imgN", [BPC, 128, NCH, F], bf16, isOutput=False)
    # hw = [W_hid.T | hidden.T] packed: one DMA -> one wait on the f32 matmul
    hw = nc.declare_dram_parameter("hw", [H, D + BPC], f32, isOutput=False)
    wimgT = nc.declare_dram_parameter("wimgT", [F, D], bf16, isOutput=False)
    wscore = nc.declare_dram_parameter("wscore", [D, 1], bf16, isOutput=False)
    ident = nc.declare_dram_parameter("ident", [128, 128], bf16, isOutput=False)
    ctx_out = nc.declare_dram_parameter("ctx", [BPC, F], f32, isOutput=True)
    wts_out = nc.declare_dram_parameter("wts", [BPC, N], f32, isOutput=True)

    with tile.TileContext(nc) as tc, ExitStack() as ctx:
        const = ctx.enter_context(tc.tile_pool(name="const", bufs=1))
        itp = ctx.enter_context(tc.tile_pool(name="imgTp", bufs=2))
        inp = ctx.enter_context(tc.tile_pool(name="imgNp", bufs=2))
        thp = ctx.enter_context(tc.tile_pool(name="tanh", bufs=6))
        scrp = ctx.enter_context(tc.tile_pool(name="scrp", bufs=1))
        wtp = ctx.enter_context(tc.tile_pool(name="wtp", bufs=1))
        exfp = ctx.enter_context(tc.tile_pool(name="exfp", bufs=1))
        wnfp = ctx.enter_context(tc.tile_pool(name="wnfp", bufs=2))
        smp = ctx.enter_context(tc.tile_pool(name="smp", bufs=2))
        csp = ctx.enter_context(tc.tile_pool(name="csb", bufs=2))
        pps = ctx.enter_context(tc.tile_pool(name="pp", bufs=3, space="PSUM"))
        scp = ctx.enter_context(tc.tile_pool(name="sc", bufs=2, space="PSUM"))
        tpp = ctx.enter_context(tc.tile_pool(name="tp", bufs=2, space="PSUM"))
        ctxps = ctx.enter_context(tc.tile_pool(name="cps", bufs=1, space="PSUM"))

        # ---- constants (consumers arranged for <=1 wait each) ----
        wimg_sb = const.tile([128, 4, D], bf16)
        nc.sync.dma_start(wimg_sb[:], wimgT.rearrange("(c p) d -> p c d", p=128))
        hw_sb = const.tile([128, 4, D + BPC], f32)
        nc.sync.dma_start(hw_sb[:], hw.rearrange("(c p) x -> p c x", p=128))
        wsc_dma = const.tile([128, 2, 1], bf16)
        nc.sync.dma_start(wsc_dma[:], wscore.rearrange("(c p) o -> p c o", p=128))
        wsc_sb = const.tile([128, 2, 1], bf16)
        nc.scalar.copy(wsc_sb[:], wsc_dma[:])
        id_sb = const.tile([128, 128], bf16)
        nc.sync.dma_start(id_sb[:], ident[:])

        # ---- proj_hidden in [d, b] layout (per-partition bias for tanh) ----
        ph_ps = tpp.tile([128, 2, BPC], f32, tag="tp")
        for dh in range(2):
            for hc in range(4):
                nc.tensor.matmul(
                    ph_ps[:, dh, :],
                    lhsT=hw_sb[:, hc, dh * 128:(dh + 1) * 128],
                    rhs=hw_sb[:, hc, D:D + BPC],
                    start=(hc == 0),
                    stop=(hc == 3),
                )
        ph_sb = const.tile([128, 2, BPC], f32)
        nc.scalar.copy(ph_sb[:], ph_ps[:])

        sc_rows = {}   # group -> [128, 8, 512] f32 score rows (4 live rows)
        wn_bf = {}     # group -> [128, NCH, 128] bf16 transposed weights
        wn_fs = {}     # group -> [128, N] f32 normalized weights (for output)
        csbs = {}      # batch -> [1, F] f32 context rows (for output)

        def phase_a_steps(b):
            """proj + tanh + score rows for one batch; yields after each sub."""
            g, k = divmod(b, 4)
            if k == 0:
                srow_new = scrp.tile([128, NNT * NSUB, 512], f32, tag="scr")
                sc_rows[g] = srow_new
            srow = sc_rows[g]
            r = 32 * k
            pend = None  # (ths, chunk-col) for deferred scores matmuls

            def flush_scores():
                nonlocal pend
                if pend is None:
                    return
                ths, cc = pend
                sc = scp.tile([128, 512], f32, tag="sc")
                for dh in range(2):
                    nc.tensor.matmul(
                        sc[r:r + 1, :],
                        lhsT=wsc_sb[:, dh, :],
                        rhs=ths[dh][:],
                        start=(dh == 0),
                        stop=(dh == 1),
                        tile_position=(0, r),
                    )
                nc.vector.tensor_copy(srow[r:r + 1, cc, :], sc[r:r + 1, :])
                pend = None

            for nt in range(NNT):
                it = itp.tile([128, 4, NT], bf16, tag="it")
                nc.sync.dma_start(it[:], imgT[b, :, :, nt * NT:(nt + 1) * NT])
                for sub in range(NSUB):
                    sl = slice(sub * 512, (sub + 1) * 512)
                    ths = []
                    for dh in range(2):
                        pp = pps.tile([128, 512], f32, tag="pp")
                        for fc in range(4):
                            nc.tensor.matmul(
                                pp[:],
                                lhsT=wimg_sb[:, fc, dh * 128:(dh + 1) * 128],
                                rhs=it[:, fc, sl],
                                start=(fc == 0),
                                stop=(fc == 3),
                            )
                        th = thp.tile([128, 512], bf16, tag="th")
                        nc.scalar.activation(
                            th[:], pp[:], AF.Tanh,
                            bias=ph_sb[:, dh, b:b + 1], scale=1.0,
                        )
                        ths.append(th)
                    flush_scores()
                    pend = (ths, nt * NSUB + sub)
                    yield
            flush_scores()

        def softmax_group(g):
            """exp/sum/normalize/transpose for 4 batches at rows {0,32,64,96}."""
            srow = sc_rows[g]
            exp_f = exfp.tile([128, N], f32, tag="expf")
            nc.scalar.activation(exp_f[:], srow[:].rearrange("p a b -> p (a b)"),
                                 AF.Exp)
            se = smp.tile([128, 1], f32, tag="se")
            nc.vector.reduce_sum(se[:], exp_f[:], axis=mybir.AxisListType.X)
            rec = smp.tile([128, 1], f32, tag="rec")
            nc.vector.reciprocal(rec[:], se[:])
            wn_f = wnfp.tile([128, N], f32, tag="wnf")
            nc.vector.tensor_scalar_mul(wn_f[:], exp_f[:], rec[:])
            wn_b = smp.tile([128, N], bf16, tag="wnb")
            nc.vector.tensor_scalar_mul(wn_b[:], exp_f[:], rec[:])
            wn_fs[g] = wn_f
            wt = wtp.tile([128, NCH, 128], bf16, tag="wt")
            wn_bf[g] = wt
            # PE-mode transposes: the PE stream is idle between groups, and
            # this avoids the ~1.2us-per-DMA_TRANSPOSE ACT sequencer cost.
            for c in range(NCH):
                tp = tpp.tile([128, 128], bf16, tag="tp")
                nc.tensor.transpose(tp[:], wn_b[:, c * 128:(c + 1) * 128],
                                    id_sb[:])
                nc.vector.tensor_copy(wt[:, c, :], tp[:])

        def phase_c_steps(g):
            """context for the 4 batches of group g, col-packed on the PE:
            the 4 M=1 matmuls per chunk land in distinct 32-col groups and
            execute concurrently in the array."""
            wt = wn_bf[g]
            cp = ctxps.tile([128, F], f32, tag="cp")
            for gg in range(NCH // CG):
                inats = []
                for k in range(4):
                    inat = inp.tile([128, CG, F], bf16, tag=f"in{k}")
                    nc.sync.dma_start(
                        inat[:], imgN[g * 4 + k, :, gg * CG:(gg + 1) * CG, :]
                    )
                    inats.append(inat)
                for j in range(CG):
                    c = gg * CG + j
                    for k in range(4):
                        nc.tensor.matmul(
                            cp[32 * k:32 * k + 1, :],
                            lhsT=wt[:, c, 32 * k:32 * k + 1],
                            rhs=inats[k][:, j, :],
                            start=(c == 0),
                            stop=(c == NCH - 1),
                            tile_position=(0, 32 * k),
                        )
                yield
            csb = csp.tile([128, F], f32, tag="csb")
            nc.vector.tensor_copy(csb[:], cp[:])
            csbs[g] = csb
            while True:
                yield

        # software pipeline: context(group 0) overlaps proj(group 1).
        # C(g0) has NCH//CG steps spread over the 32 A-subs of group 1.
        for b in range(4):
            for _ in phase_a_steps(b):
                pass
        softmax_group(0)
        cgen = phase_c_steps(0)
        csteps_left = NCH // CG + 1
        sub_i = 0
        for k in range(4):
            for _ in phase_a_steps(4 + k):
                sub_i += 1
                if sub_i % 4 == 0 and csteps_left > 0:
                    next(cgen)
                    csteps_left -= 1
        while csteps_left > 0:
            next(cgen)
            csteps_left -= 1
        softmax_group(1)
        cgen = phase_c_steps(1)
        for _ in range(NCH // CG + 1):
            next(cgen)

        # all output DMAs at the end (ACT ring; SP ring stays pure input)
        for g in range(2):
            for kk in range(4):
                nc.scalar.dma_start(wts_out[g * 4 + kk],
                                    wn_fs[g][32 * kk:32 * kk + 1, :])
                nc.scalar.dma_start(ctx_out[g * 4 + kk:g * 4 + kk + 1, :],
                                    csbs[g][32 * kk:32 * kk + 1, :])

    nc.compile()
    _nc_cache["nc"] = nc
    return nc


def _in_maps(image_features, hidden_state, W_img, W_hid, W_score):
    img_bf = image_features.astype(_BF)                        # [64, 4096, 512]
    # context rhs: [B, p, n-chunk, f] with f contiguous per (p, chunk)
    imgN_q = np.ascontiguousarray(
        img_bf.reshape(B, NCH, 128, F).transpose(0, 2, 1, 3)
    )                                                          # [64, 128, 32, 512]
    # proj rhs: [B, p, f-chunk, n] with n contiguous per (p, chunk)
    imgT_bf = img_bf.transpose(0, 2, 1)                        # [64, 512, 4096] view
    imgT_q = np.ascontiguousarray(
        imgT_bf.reshape(B, 4, 128, N).transpose(0, 2, 1, 3)
    )                                                          # [64, 128, 4, 4096]
    wimgT = np.ascontiguousarray(W_img.T).astype(_BF)          # [512, 256]
    whidT = W_hid.T.astype(np.float32)                         # [512, 256]
    wsc = np.ascontiguousarray(W_score.reshape(1, D).T).astype(_BF)  # [256, 1]
    eye = np.eye(128, dtype=np.float32).astype(_BF)

    in_maps = []
    for c in range(NCORES):
        s = slice(c * BPC, (c + 1) * BPC)
        hwpack = np.concatenate(
            [whidT, hidden_state[s].T.astype(np.float32)], axis=1
        )  # [512, 264]
        in_maps.append({
            "imgT": imgT_q[s],
            "imgN": imgN_q[s],
            "hw": np.ascontiguousarray(hwpack),
            "wimgT": wimgT,
            "wscore": wsc,
            "ident": eye,
        })
    return in_maps


def kernel(image_features, hidden_state, W_img, W_hid, W_score):
    from concourse.bass_utils import run_bass_kernel_spmd

    nc = _build_nc()
    in_maps = _in_maps(image_features, hidden_state, W_img, W_hid, W_score)
    res = run_bass_kernel_spmd(nc, in_maps, list(range(NCORES))).results
    ctx = np.concatenate([r["ctx"] for r in res], axis=0)
    wts = np.concatenate([r["wts"] for r in res], axis=0)
    return (ctx, wts)


# revision 56
# speedup vs baseline: 1.2954x; 1.0442x over previous
"""Additive attention (Bahdanau) Trainium2 Bass kernel.

Sharding: data-parallel over batch B=64 -> 8 cores x 8 batches.
Per core, batches processed in 2 groups of 4 (score rows live at PSUM
partitions {0,32,64,96}):
  proj[d, n] = W_img @ img[b, n, :]           PE, [d,n] tiles, lhsT=W_imgT
  tanh tiles = tanh(proj + ph[d])             ACT, per-partition bias fused
  score row [1, 512] = W_score . tanh         PE, static [128,1] lhsT (row b%4*32)
  group softmax: exp/sum/normalize            ACT+DVE on [128, 4096] group tile
  w columns for context via DMA-xbar transpose of bf16 weights
  context = sum_n w[n] * img[b, n, :]         PE, lhsT = w column [128,1]

Host feeds partition-tiled layouts so every DMA has 4 KiB contiguous runs:
  imgT_q [8, 128, 4, 4096]  (p, f-chunk, n)  proj rhs tiles
  imgN_q [8, 128, 32, 512]  (p, n-chunk, f)  context rhs tiles

TRN2 instructions embed only ONE sync wait; Bacc's generate_event_semaphores
legalizes the rest, and ops are arranged to keep waits rare. Emission order
software-pipelines context(group 0) against proj(group 1).
"""

import sys
import numpy as np

for p in ("/opt/trn_rl_repo",):
    if p not in sys.path:
        sys.path.insert(0, p)

import ml_dtypes

B, N, F, H, D = 64, 4096, 512, 512, 256
NCORES = 8
BPC = B // NCORES  # batches per core
NT = 2048          # n-tile size for proj loads
NNT = N // NT      # 2 tiles
NSUB = NT // 512   # 4 x 512 matmul slices per tile
NCH = N // 128     # 32 n-chunks
CG = 4             # context chunks per load
_BF = ml_dtypes.bfloat16

_nc_cache = {}


def _build_nc():
    if "nc" in _nc_cache:
        return _nc_cache["nc"]
    from contextlib import ExitStack

    import concourse.bass as bass  # noqa: F401
    import concourse.tile as tile
    from concourse import bacc, mybir

    bf16 = mybir.dt.bfloat16
    f32 = mybir.dt.float32
    AF = mybir.ActivationFunctionType

    nc = bacc.Bacc("TRN2")

    imgT = nc.declare_dram_parameter("imgT", [BPC, 128, 4, N], bf16, isOutput=False)
    imgN = nc.declare_dram_parameter("imgN", [BPC, 128, NCH, F], bf16, isOutput=False)
    # hw = [W_hid.T | hidden.T] packed: one DMA -> one wait on the f32 matmul
    hw = nc.declare_dram_parameter("hw", [H, D + BPC], f32, isOutput=False)
    wimgT = nc.declare_dram_parameter("wimgT", [F, D], bf16, isOutput=False)
    wscore = nc.declare_dram_parameter("wscore", [D, 1], bf16, isOutput=False)
    ident = nc.declare_dram_parameter("ident", [128, 128], bf16, isOutput=False)
    ctx_out = nc.declare_dram_parameter("ctx", [BPC, F], f32, isOutput=True)
    wts_out = nc.declare_dram_parameter("wts", [BPC, N], f32, isOutput=True)

    with tile.TileContext(nc) as tc, ExitStack() as ctx:
        const = ctx.enter_context(tc.tile_pool(name="const", bufs=1))
        itp = ctx.enter_context(tc.tile_pool(name="imgTp", bufs=2))
        inp = ctx.enter_context(tc.tile_pool(name="imgNp", bufs=2))
        thp = ctx.enter_context(tc.tile_pool(name="tanh", bufs=6))
        scrp = ctx.enter_context(tc.tile_pool(name="scrp", bufs=1))
        wtp = ctx.enter_context(tc.tile_pool(name="wtp", bufs=1))
        exfp = ctx.enter_context(tc.tile_pool(name="exfp", bufs=1))
        wnfp = ctx.enter_context(tc.tile_pool(name="wnfp", bufs=2))
        smp = ctx.enter_context(tc.tile_pool(name="smp", bufs=2))
        csp = ctx.enter_context(tc.tile_pool(name="csb", bufs=2))
        pps = ctx.enter_context(tc.tile_pool(name="pp", bufs=3, space="PSUM"))
        scp = ctx.enter_context(tc.tile_pool(name="sc", bufs=2, space="PSUM"))
        tpp = ctx.enter_context(tc.tile_pool(name="tp", bufs=2, space="PSUM"))
        ctxps = ctx.enter_context(tc.tile_pool(name="cps", bufs=1, space="PSUM"))

        # first img tiles before anything else so the HBM stream starts at t=0
        prefetched = {}
        for _nt in range(NNT):
            pf_it = itp.tile([128, 4, NT], bf16, tag="it")
            nc.sync.dma_start(pf_it[:], imgT[0, :, :, _nt * NT:(_nt + 1) * NT])
            prefetched[(0, _nt)] = pf_it

        # ---- constants (consumers arranged for <=1 wait each) ----
        wimg_sb = const.tile([128, 4, D], bf16)
        nc.sync.dma_start(wimg_sb[:], wimgT.rearrange("(c p) d -> p c d", p=128))
        hw_sb = const.tile([128, 4, D + BPC], f32)
        nc.sync.dma_start(hw_sb[:], hw.rearrange("(c p) x -> p c x", p=128))
        wsc_dma = const.tile([128, 2, 1], bf16)
        nc.sync.dma_start(wsc_dma[:], wscore.rearrange("(c p) o -> p c o", p=128))
        wsc_sb = const.tile([128, 2, 1], bf16)
        nc.scalar.copy(wsc_sb[:], wsc_dma[:])
        id_sb = const.tile([128, 128], bf16)
        nc.sync.dma_start(id_sb[:], ident[:])

        # ---- proj_hidden in [d, b] layout (per-partition bias for tanh) ----
        ph_ps = tpp.tile([128, 2, BPC], f32, tag="tp")
        for dh in range(2):
            for hc in range(4):
                nc.tensor.matmul(
                    ph_ps[:, dh, :],
                    lhsT=hw_sb[:, hc, dh * 128:(dh + 1) * 128],
                    rhs=hw_sb[:, hc, D:D + BPC],
                    start=(hc == 0),
                    stop=(hc == 3),
                )
        ph_sb = const.tile([128, 2, BPC], f32)
        nc.scalar.copy(ph_sb[:], ph_ps[:])

        sc_rows = {}   # group -> [128, 8, 512] f32 score rows (4 live rows)
        wn_bf = {}     # group -> [128, NCH, 128] bf16 transposed weights
        wn_fs = {}     # group -> [128, N] f32 normalized weights (for output)
        csbs = {}      # batch -> [1, F] f32 context rows (for output)

        def phase_a_steps(b):
            """proj + tanh + score rows for one batch; yields after each sub."""
            g, k = divmod(b, 4)
            if k == 0:
                srow_new = scrp.tile([128, NNT * NSUB, 512], f32, tag="scr")
                sc_rows[g] = srow_new
            srow = sc_rows[g]
            r = 32 * k
            pend = None  # (ths, chunk-col) for deferred scores matmuls

            def flush_scores():
                nonlocal pend
                if pend is None:
                    return
                ths, cc = pend
                sc = scp.tile([128, 512], f32, tag="sc")
                for dh in range(2):
                    nc.tensor.matmul(
                        sc[r:r + 1, :],
                        lhsT=wsc_sb[:, dh, :],
                        rhs=ths[dh][:],
                        start=(dh == 0),
                        stop=(dh == 1),
                        tile_position=(0, r),
                    )
                nc.vector.tensor_copy(srow[r:r + 1, cc, :], sc[r:r + 1, :])
                pend = None

            for nt in range(NNT):
                it = prefetched.pop((b, nt), None)
                if it is None:
                    it = itp.tile([128, 4, NT], bf16, tag="it")
                    nc.sync.dma_start(it[:], imgT[b, :, :, nt * NT:(nt + 1) * NT])
                for sub in range(NSUB):
                    sl = slice(sub * 512, (sub + 1) * 512)
                    ths = []
                    for dh in range(2):
                        pp = pps.tile([128, 512], f32, tag="pp")
                        for fc in range(4):
                            nc.tensor.matmul(
                                pp[:],
                                lhsT=wimg_sb[:, fc, dh * 128:(dh + 1) * 128],
                                rhs=it[:, fc, sl],
                                start=(fc == 0),
                                stop=(fc == 3),
                            )
                        th = thp.tile([128, 512], bf16, tag="th")
                        nc.scalar.activation(
                            th[:], pp[:], AF.Tanh,
                            bias=ph_sb[:, dh, b:b + 1], scale=1.0,
                        )
                        ths.append(th)
                    flush_scores()
                    pend = (ths, nt * NSUB + sub)
                    yield
            flush_scores()

        def softmax_group(g):
            """exp/sum/normalize/transpose for 4 batches at rows {0,32,64,96}."""
            srow = sc_rows[g]
            exp_f = exfp.tile([128, N], f32, tag="expf")
            se = smp.tile([128, 1], f32, tag="se")
            # exp with fused free-dim sum: one ACT op feeds both paths
            nc.scalar.activation(exp_f[:], srow[:].rearrange("p a b -> p (a b)"),
                                 AF.Exp, accum_out=se[:])
            rec = smp.tile([128, 1], f32, tag="rec")
            nc.vector.reciprocal(rec[:], se[:])
            wt = wtp.tile([128, NCH, 128], bf16, tag="wt")
            wn_bf[g] = wt
            # normalize + transpose per chunk so the first context matmul
            # unblocks early; transposes ride the otherwise-idle PE.
            for c in range(NCH):
                wn_c = smp.tile([128, 128], bf16, tag="wnc")
                nc.vector.tensor_scalar_mul(
                    wn_c[:], exp_f[:, c * 128:(c + 1) * 128], rec[:]
                )
                tp = tpp.tile([128, 128], bf16, tag="tp")
                nc.tensor.transpose(tp[:], wn_c[:], id_sb[:])
                nc.vector.tensor_copy(wt[:, c, :], tp[:])
            wn_f = wnfp.tile([128, N], f32, tag="wnf")
            nc.vector.tensor_scalar_mul(wn_f[:], exp_f[:], rec[:])
            wn_fs[g] = wn_f

        def phase_c_steps(g):
            """context for the 4 batches of group g, col-packed on the PE:
            the 4 M=1 matmuls per chunk land in distinct 32-col groups and
            execute concurrently in the array."""
            wt = wn_bf[g]
            cp = ctxps.tile([128, F], f32, tag="cp")
            for gg in range(NCH // CG):
                inats = []
                for k in range(4):
                    inat = inp.tile([128, CG, F], bf16, tag=f"in{k}")
                    nc.sync.dma_start(
                        inat[:], imgN[g * 4 + k, :, gg * CG:(gg + 1) * CG, :]
                    )
                    inats.append(inat)
                for j in range(CG):
                    c = gg * CG + j
                    for k in range(4):
                        nc.tensor.matmul(
                            cp[32 * k:32 * k + 1, :],
                            lhsT=wt[:, c, 32 * k:32 * k + 1],
                            rhs=inats[k][:, j, :],
                            start=(c == 0),
                            stop=(c == NCH - 1),
                            tile_position=(0, 32 * k),
                        )
                yield
            csb = csp.tile([128, F], f32, tag="csb")
            nc.vector.tensor_copy(csb[:], cp[:])
            csbs[g] = csb
            while True:
                yield

        # software pipeline: context(group 0) overlaps proj(group 1).
        # C(g0) has NCH//CG steps spread over the 32 A-subs of group 1.
        for b in range(4):
            for _ in phase_a_steps(b):
                pass
        softmax_group(0)
        cgen = phase_c_steps(0)
        csteps_left = NCH // CG + 1
        sub_i = 0
        for k in range(4):
            for _ in phase_a_steps(4 + k):
                sub_i += 1
                if sub_i % 4 == 0 and csteps_left > 0:
                    next(cgen)
                    csteps_left -= 1
        while csteps_left > 0:
            next(cgen)
            csteps_left -= 1
        softmax_group(1)
        cgen = phase_c_steps(1)
        for _ in range(NCH // CG + 1):
            next(cgen)

        # all output DMAs at the end (ACT ring; SP ring stays pure input)
        for g in range(2):
            for kk in range(4):
                nc.scalar.dma_start(wts_out[g * 4 + kk],
                                    wn_fs[g][32 * kk:32 * kk + 1, :])
                nc.scalar.dma_start(ctx_out[g * 4 + kk:g * 4 + kk + 1, :],
                                    csbs[g][32 * kk:32 * kk + 1, :])

    nc.compile()
    _nc_cache["nc"] = nc
    return nc


def _in_maps(image_features, hidden_state, W_img, W_hid, W_score):
    img_bf = image_features.astype(_BF)                        # [64, 4096, 512]
    # context rhs: [B, p, n-chunk, f] with f contiguous per (p, chunk)
    imgN_q = np.ascontiguousarray(
        img_bf.reshape(B, NCH, 128, F).transpose(0, 2, 1, 3)
    )                                                          # [64, 128, 32, 512]
    # proj rhs: [B, p, f-chunk, n] with n contiguous per (p, chunk)
    imgT_bf = img_bf.transpose(0, 2, 1)                        # [64, 512, 4096] view
    imgT_q = np.ascontiguousarray(
        imgT_bf.reshape(B, 4, 128, N).transpose(0, 2, 1, 3)
    )                                                          # [64, 128, 4, 4096]
    wimgT = np.ascontiguousarray(W_img.T).astype(_BF)          # [512, 256]
    whidT = W_hid.T.astype(np.float32)                         # [512, 256]
    wsc = np.ascontiguousarray(W_score.reshape(1, D).T).astype(_BF)  # [256, 1]
    eye = np.eye(128, dtype=np.float32).astype(_BF)

    in_maps = []
    for c in range(NCORES):
        s = slice(c * BPC, (c + 1) * BPC)
        hwpack = np.concatenate(
            [whidT, hidden_state[s].T.astype(np.float32)], axis=1
        )  # [512, 264]
        in_maps.append({
            "imgT": imgT_q[s],
            "imgN": imgN_q[s],
            "hw": np.ascontiguousarray(hwpack),
            "wimgT": wimgT,
            "wscore": wsc,
            "ident": eye,
        })
    return in_maps


def kernel(image_features, hidden_state, W_img, W_hid, W_score):
    from concourse.bass_utils import run_bass_kernel_spmd

    nc = _build_nc()
    in_maps = _in_maps(image_features, hidden_state, W_img, W_hid, W_score)
    res = run_bass_kernel_spmd(nc, in_maps, list(range(NCORES))).results
    ctx = np.concatenate([r["ctx"] for r in res], axis=0)
    wts = np.concatenate([r["wts"] for r in res], axis=0)
    return (ctx, wts)


# revision 58
# speedup vs baseline: 1.3365x; 1.0317x over previous
"""Additive attention (Bahdanau) Trainium2 Bass kernel.

Sharding: data-parallel over batch B=64 -> 8 cores x 8 batches.
Per core, batches processed in 2 groups of 4 (score rows live at PSUM
partitions {0,32,64,96}):
  proj[d, n] = W_img @ img[b, n, :]           PE, [d,n] tiles, lhsT=W_imgT
  tanh tiles = tanh(proj + ph[d])             ACT, per-partition bias fused
  score row [1, 512] = W_score . tanh         PE, static [128,1] lhsT (row b%4*32)
  group softmax: exp/sum/normalize            ACT+DVE on [128, 4096] group tile
  w columns for context via DMA-xbar transpose of bf16 weights
  context = sum_n w[n] * img[b, n, :]         PE, lhsT = w column [128,1]

Host feeds partition-tiled layouts so every DMA has 4 KiB contiguous runs:
  imgT_q [8, 128, 4, 4096]  (p, f-chunk, n)  proj rhs tiles
  imgN_q [8, 128, 32, 512]  (p, n-chunk, f)  context rhs tiles

TRN2 instructions embed only ONE sync wait; Bacc's generate_event_semaphores
legalizes the rest, and ops are arranged to keep waits rare. Emission order
software-pipelines context(group 0) against proj(group 1).
"""

import sys
import numpy as np

for p in ("/opt/trn_rl_repo",):
    if p not in sys.path:
        sys.path.insert(0, p)

import ml_dtypes

B, N, F, H, D = 64, 4096, 512, 512, 256
NCORES = 8
BPC = B // NCORES  # batches per core
NT = 2048          # n-tile size for proj loads
NNT = N // NT      # 2 tiles
NSUB = NT // 512   # 4 x 512 matmul slices per tile
NCH = N // 128     # 32 n-chunks
CG = 4             # context chunks per load
_BF = ml_dtypes.bfloat16

_nc_cache = {}


def _build_nc():
    if "nc" in _nc_cache:
        return _nc_cache["nc"]
    from contextlib import ExitStack

    import concourse.bass as bass  # noqa: F401
    import concourse.tile as tile
    from concourse import bacc, mybir

    bf16 = mybir.dt.bfloat16
    f32 = mybir.dt.float32
    AF = mybir.ActivationFunctionType

    nc = bacc.Bacc("TRN2")

    imgT = nc.declare_dram_parameter("imgT", [BPC, 128, 4, N], bf16, isOutput=False)
    imgN = nc.declare_dram_parameter("imgN", [BPC, 128, NCH, F], bf16, isOutput=False)
    # hw = [W_hid.T | hidden.T] packed: one DMA -> one wait on the f32 matmul
    hw = nc.declare_dram_parameter("hw", [H, D + BPC], f32, isOutput=False)
    wimgT = nc.declare_dram_parameter("wimgT", [F, D], bf16, isOutput=False)
    wscore = nc.declare_dram_parameter("wscore", [D, 1], bf16, isOutput=False)
    ident = nc.declare_dram_parameter("ident", [128, 128], bf16, isOutput=False)
    ctx_out = nc.declare_dram_parameter("ctx", [BPC, F], f32, isOutput=True)
    wts_out = nc.declare_dram_parameter("wts", [BPC, N], f32, isOutput=True)

    with tile.TileContext(nc) as tc, ExitStack() as ctx:
        const = ctx.enter_context(tc.tile_pool(name="const", bufs=1))
        itp = ctx.enter_context(tc.tile_pool(name="imgTp", bufs=2))
        inp = ctx.enter_context(tc.tile_pool(name="imgNp", bufs=2))
        thp = ctx.enter_context(tc.tile_pool(name="tanh", bufs=6))
        scrp = ctx.enter_context(tc.tile_pool(name="scrp", bufs=1))
        wtp = ctx.enter_context(tc.tile_pool(name="wtp", bufs=1))
        exfp = ctx.enter_context(tc.tile_pool(name="exfp", bufs=1))
        wnfp = ctx.enter_context(tc.tile_pool(name="wnfp", bufs=2))
        smp = ctx.enter_context(tc.tile_pool(name="smp", bufs=2))
        csp = ctx.enter_context(tc.tile_pool(name="csb", bufs=2))
        pps = ctx.enter_context(tc.tile_pool(name="pp", bufs=3, space="PSUM"))
        scp = ctx.enter_context(tc.tile_pool(name="sc", bufs=2, space="PSUM"))
        tpp = ctx.enter_context(tc.tile_pool(name="tp", bufs=2, space="PSUM"))
        ctxps = ctx.enter_context(tc.tile_pool(name="cps", bufs=1, space="PSUM"))

        prefetched = {}

        # ---- constants (consumers arranged for <=1 wait each) ----
        wimg_sb = const.tile([128, 4, D], bf16)
        nc.sync.dma_start(wimg_sb[:], wimgT.rearrange("(c p) d -> p c d", p=128))
        hw_sb = const.tile([128, 4, D + BPC], f32)
        nc.sync.dma_start(hw_sb[:], hw.rearrange("(c p) x -> p c x", p=128))
        wsc_dma = const.tile([128, 2, 1], bf16)
        nc.sync.dma_start(wsc_dma[:], wscore.rearrange("(c p) o -> p c o", p=128))
        wsc_sb = const.tile([128, 2, 1], bf16)
        nc.scalar.copy(wsc_sb[:], wsc_dma[:])
        id_sb = const.tile([128, 128], bf16)
        nc.sync.dma_start(id_sb[:], ident[:])

        # img prefetch right after the (tiny) consts so PE unblocks ASAP
        for _nt in range(NNT):
            pf_it = itp.tile([128, 4, NT], bf16, tag="it")
            nc.sync.dma_start(pf_it[:], imgT[0, :, :, _nt * NT:(_nt + 1) * NT])
            prefetched[(0, _nt)] = pf_it

        # ---- proj_hidden in [d, b] layout (per-partition bias for tanh) ----
        ph_ps = tpp.tile([128, 2, BPC], f32, tag="tp")
        for dh in range(2):
            for hc in range(4):
                nc.tensor.matmul(
                    ph_ps[:, dh, :],
                    lhsT=hw_sb[:, hc, dh * 128:(dh + 1) * 128],
                    rhs=hw_sb[:, hc, D:D + BPC],
                    start=(hc == 0),
                    stop=(hc == 3),
                )
        ph_sb = const.tile([128, 2, BPC], f32)
        nc.scalar.copy(ph_sb[:], ph_ps[:])

        sc_rows = {}   # group -> [128, 8, 512] f32 score rows (4 live rows)
        wn_bf = {}     # group -> [128, NCH, 128] bf16 transposed weights
        wn_fs = {}     # group -> [128, N] f32 normalized weights (for output)
        csbs = {}      # batch -> [1, F] f32 context rows (for output)

        def phase_a_steps(b):
            """proj + tanh + score rows for one batch; yields after each sub."""
            g, k = divmod(b, 4)
            if k == 0:
                srow_new = scrp.tile([128, NNT * NSUB, 512], f32, tag="scr")
                sc_rows[g] = srow_new
            srow = sc_rows[g]
            r = 32 * k
            pend = None  # (ths, chunk-col) for deferred scores matmuls

            def flush_scores():
                nonlocal pend
                if pend is None:
                    return
                ths, cc = pend
                sc = scp.tile([128, 512], f32, tag="sc")
                for dh in range(2):
                    nc.tensor.matmul(
                        sc[r:r + 1, :],
                        lhsT=wsc_sb[:, dh, :],
                        rhs=ths[dh][:],
                        start=(dh == 0),
                        stop=(dh == 1),
                        tile_position=(0, r),
                    )
                nc.vector.tensor_copy(srow[r:r + 1, cc, :], sc[r:r + 1, :])
                pend = None

            for nt in range(NNT):
                it = prefetched.pop((b, nt), None)
                if it is None:
                    it = itp.tile([128, 4, NT], bf16, tag="it")
                    nc.sync.dma_start(it[:], imgT[b, :, :, nt * NT:(nt + 1) * NT])
                for sub in range(NSUB):
                    sl = slice(sub * 512, (sub + 1) * 512)
                    ths = []
                    for dh in range(2):
                        pp = pps.tile([128, 512], f32, tag="pp")
                        for fc in range(4):
                            nc.tensor.matmul(
                                pp[:],
                                lhsT=wimg_sb[:, fc, dh * 128:(dh + 1) * 128],
                                rhs=it[:, fc, sl],
                                start=(fc == 0),
                                stop=(fc == 3),
                            )
                        th = thp.tile([128, 512], bf16, tag="th")
                        nc.scalar.activation(
                            th[:], pp[:], AF.Tanh,
                            bias=ph_sb[:, dh, b:b + 1], scale=1.0,
                        )
                        ths.append(th)
                    flush_scores()
                    pend = (ths, nt * NSUB + sub)
                    yield
            flush_scores()

        def softmax_group(g):
            """exp/sum/normalize/transpose for 4 batches at rows {0,32,64,96}."""
            srow = sc_rows[g]
            exp_f = exfp.tile([128, N], f32, tag="expf")
            se = smp.tile([128, 1], f32, tag="se")
            # exp with fused free-dim sum: one ACT op feeds both paths
            nc.scalar.activation(exp_f[:], srow[:].rearrange("p a b -> p (a b)"),
                                 AF.Exp, accum_out=se[:])
            rec = smp.tile([128, 1], f32, tag="rec")
            nc.vector.reciprocal(rec[:], se[:])
            wt = wtp.tile([128, NCH, 128], bf16, tag="wt")
            wn_bf[g] = wt
            # normalize + transpose per chunk so the first context matmul
            # unblocks early; transposes ride the otherwise-idle PE.
            for c in range(NCH):
                wn_c = smp.tile([128, 128], bf16, tag="wnc")
                nc.vector.tensor_scalar_mul(
                    wn_c[:], exp_f[:, c * 128:(c + 1) * 128], rec[:]
                )
                tp = tpp.tile([128, 128], bf16, tag="tp")
                nc.tensor.transpose(tp[:], wn_c[:], id_sb[:])
                nc.vector.tensor_copy(wt[:, c, :], tp[:])
            wn_f = wnfp.tile([128, N], f32, tag="wnf")
            nc.vector.tensor_scalar_mul(wn_f[:], exp_f[:], rec[:])
            wn_fs[g] = wn_f

        def phase_c_steps(g):
            """context for the 4 batches of group g, col-packed on the PE:
            the 4 M=1 matmuls per chunk land in distinct 32-col groups and
            execute concurrently in the array."""
            wt = wn_bf[g]
            cp = ctxps.tile([128, F], f32, tag="cp")
            for gg in range(NCH // CG):
                inats = []
                for k in range(4):
                    inat = inp.tile([128, CG, F], bf16, tag=f"in{k}")
                    nc.sync.dma_start(
                        inat[:], imgN[g * 4 + k, :, gg * CG:(gg + 1) * CG, :]
                    )
                    inats.append(inat)
                for j in range(CG):
                    c = gg * CG + j
                    for k in range(4):
                        nc.tensor.matmul(
                            cp[32 * k:32 * k + 1, :],
                            lhsT=wt[:, c, 32 * k:32 * k + 1],
                            rhs=inats[k][:, j, :],
                            start=(c == 0),
                            stop=(c == NCH - 1),
                            tile_position=(0, 32 * k),
                        )
                yield
            csb = csp.tile([128, F], f32, tag="csb")
            nc.vector.tensor_copy(csb[:], cp[:])
            csbs[g] = csb
            while True:
                yield

        # software pipeline: context(group 0) overlaps proj(group 1).
        # C(g0) has NCH//CG steps spread over the 32 A-subs of group 1.
        for b in range(4):
            for _ in phase_a_steps(b):
                pass
        softmax_group(0)
        cgen = phase_c_steps(0)
        csteps_left = NCH // CG + 1
        sub_i = 0
        for k in range(4):
            for _ in phase_a_steps(4 + k):
                sub_i += 1
                if sub_i % 4 == 0 and csteps_left > 0:
                    next(cgen)
                    csteps_left -= 1
        while csteps_left > 0:
            next(cgen)
            csteps_left -= 1
        softmax_group(1)
        cgen = phase_c_steps(1)
        for _ in range(NCH // CG + 1):
            next(cgen)

        # all output DMAs at the end (ACT ring; SP ring stays pure input)
        for g in range(2):
            for kk in range(4):
                nc.scalar.dma_start(wts_out[g * 4 + kk],
                                    wn_fs[g][32 * kk:32 * kk + 1, :])
                nc.scalar.dma_start(ctx_out[g * 4 + kk:g * 4 + kk + 1, :],
                                    csbs[g][32 * kk:32 * kk + 1, :])

    nc.compile()
    _nc_cache["nc"] = nc
    return nc


def _in_maps(image_features, hidden_state, W_img, W_hid, W_score):
    img_bf = image_features.astype(_BF)                        # [64, 4096, 512]
    # context rhs: [B, p, n-chunk, f] with f contiguous per (p, chunk)
    imgN_q = np.ascontiguousarray(
        img_bf.reshape(B, NCH, 128, F).transpose(0, 2, 1, 3)
    )                                                          # [64, 128, 32, 512]
    # proj rhs: [B, p, f-chunk, n] with n contiguous per (p, chunk)
    imgT_bf = img_bf.transpose(0, 2, 1)                        # [64, 512, 4096] view
    imgT_q = np.ascontiguousarray(
        imgT_bf.reshape(B, 4, 128, N).transpose(0, 2, 1, 3)
    )                                                          # [64, 128, 4, 4096]
    wimgT = np.ascontiguousarray(W_img.T).astype(_BF)          # [512, 256]
    whidT = W_hid.T.astype(np.float32)                         # [512, 256]
    wsc = np.ascontiguousarray(W_score.reshape(1, D).T).astype(_BF)  # [256, 1]
    eye = np.eye(128, dtype=np.float32).astype(_BF)

    in_maps = []
    for c in range(NCORES):
        s = slice(c * BPC, (c + 1) * BPC)
        hwpack = np.concatenate(
            [whidT, hidden_state[s].T.astype(np.float32)], axis=1
        )  # [512, 264]
        in_maps.append({
            "imgT": imgT_q[s],
            "imgN": imgN_q[s],
            "hw": np.ascontiguousarray(hwpack),
            "wimgT": wimgT,
            "wscore": wsc,
            "ident": eye,
        })
    return in_maps


def kernel(image_features, hidden_state, W_img, W_hid, W_score):
    from concourse.bass_utils import run_bass_kernel_spmd

    nc = _build_nc()
    in_maps = _in_maps(image_features, hidden_state, W_img, W_hid, W_score)
    res = run_bass_kernel_spmd(nc, in_maps, list(range(NCORES))).results
    ctx = np.concatenate([r["ctx"] for r in res], axis=0)
    wts = np.concatenate([r["wts"] for r in res], axis=0)
    return (ctx, wts)


# revision 60
# speedup vs baseline: 1.5577x; 1.1655x over previous
"""Additive attention (Bahdanau) Trainium2 Bass kernel.

Sharding: data-parallel over batch B=64 -> 8 cores x 8 batches.
Per core, batches processed in 2 groups of 4 (score rows live at PSUM
partitions {0,32,64,96}):
  proj[d, n] = W_img @ img[b, n, :]           PE, [d,n] tiles, lhsT=W_imgT
  tanh tiles = tanh(proj + ph[d])             ACT, per-partition bias fused
  score row [1, 512] = W_score . tanh         PE, static [128,1] lhsT (row b%4*32)
  group softmax: exp/sum/normalize            ACT+DVE on [128, 4096] group tile
  w columns for context via DMA-xbar transpose of bf16 weights
  context = sum_n w[n] * img[b, n, :]         PE, lhsT = w column [128,1]

Host feeds partition-tiled layouts so every DMA has 4 KiB contiguous runs:
  imgT_q [8, 128, 4, 4096]  (p, f-chunk, n)  proj rhs tiles
  imgN_q [8, 128, 32, 512]  (p, n-chunk, f)  context rhs tiles

TRN2 instructions embed only ONE sync wait; Bacc's generate_event_semaphores
legalizes the rest, and ops are arranged to keep waits rare. Emission order
software-pipelines context(group 0) against proj(group 1).
"""

import sys
import numpy as np

for p in ("/opt/trn_rl_repo",):
    if p not in sys.path:
        sys.path.insert(0, p)

import ml_dtypes

B, N, F, H, D = 64, 4096, 512, 512, 256
NCORES = 8
BPC = B // NCORES  # batches per core
NT = 2048          # n-tile size for proj loads
NNT = N // NT      # 2 tiles
NSUB = NT // 512   # 4 x 512 matmul slices per tile
NCH = N // 128     # 32 n-chunks
CG = 4             # context chunks per load
_BF = ml_dtypes.bfloat16

_nc_cache = {}


def _build_nc():
    if "nc" in _nc_cache:
        return _nc_cache["nc"]
    from contextlib import ExitStack

    import concourse.bass as bass  # noqa: F401
    import concourse.tile as tile
    from concourse import bacc, mybir

    bf16 = mybir.dt.bfloat16
    f32 = mybir.dt.float32
    AF = mybir.ActivationFunctionType

    nc = bacc.Bacc("TRN2")

    imgT = nc.declare_dram_parameter("imgT", [BPC, 128, 4, N], bf16, isOutput=False)
    imgN = nc.declare_dram_parameter("imgN", [BPC, 128, NCH, F], bf16, isOutput=False)
    # hw = [W_hid.T | hidden.T] packed: one DMA -> one wait on the f32 matmul
    hw = nc.declare_dram_parameter("hw", [H, D + BPC], f32, isOutput=False)
    wimgT = nc.declare_dram_parameter("wimgT", [F, D], bf16, isOutput=False)
    wscore = nc.declare_dram_parameter("wscore", [D, 1], bf16, isOutput=False)
    ident = nc.declare_dram_parameter("ident", [128, 128], bf16, isOutput=False)
    ctx_out = nc.declare_dram_parameter("ctx", [BPC, F], f32, isOutput=True)
    wts_out = nc.declare_dram_parameter("wts", [BPC, N], f32, isOutput=True)

    with tile.TileContext(nc) as tc, ExitStack() as ctx:
        const = ctx.enter_context(tc.tile_pool(name="const", bufs=1))
        itp = ctx.enter_context(tc.tile_pool(name="imgTp", bufs=3))
        inp = ctx.enter_context(tc.tile_pool(name="imgNp", bufs=2))
        thp = ctx.enter_context(tc.tile_pool(name="tanh", bufs=6))
        scrp = ctx.enter_context(tc.tile_pool(name="scrp", bufs=1))
        wtp = ctx.enter_context(tc.tile_pool(name="wtp", bufs=1))
        exfp = ctx.enter_context(tc.tile_pool(name="exfp", bufs=1))
        wnfp = ctx.enter_context(tc.tile_pool(name="wnfp", bufs=2))
        smp = ctx.enter_context(tc.tile_pool(name="smp", bufs=2))
        csp = ctx.enter_context(tc.tile_pool(name="csb", bufs=2))
        pps = ctx.enter_context(tc.tile_pool(name="pp", bufs=3, space="PSUM"))
        scp = ctx.enter_context(tc.tile_pool(name="sc", bufs=2, space="PSUM"))
        tpp = ctx.enter_context(tc.tile_pool(name="tp", bufs=2, space="PSUM"))
        ctxps = ctx.enter_context(tc.tile_pool(name="cps", bufs=1, space="PSUM"))

        prefetched = {}

        # ---- constants (consumers arranged for <=1 wait each) ----
        wimg_sb = const.tile([128, 4, D], bf16)
        nc.sync.dma_start(wimg_sb[:], wimgT.rearrange("(c p) d -> p c d", p=128))
        hw_sb = const.tile([128, 4, D + BPC], f32)
        nc.sync.dma_start(hw_sb[:], hw.rearrange("(c p) x -> p c x", p=128))
        wsc_dma = const.tile([128, 2, 1], bf16)
        nc.sync.dma_start(wsc_dma[:], wscore.rearrange("(c p) o -> p c o", p=128))
        wsc_sb = const.tile([128, 2, 1], bf16)
        nc.scalar.copy(wsc_sb[:], wsc_dma[:])
        id_sb = const.tile([128, 128], bf16)
        nc.sync.dma_start(id_sb[:], ident[:])

        # img prefetch right after the (tiny) consts so PE unblocks ASAP;
        # the very first tile arrives per f-chunk so fc=0 matmuls start early
        pf_it = itp.tile([128, 4, NT], bf16, tag="it")
        for _fc in range(4):
            nc.sync.dma_start(pf_it[:, _fc, :], imgT[0, :, _fc, 0:NT])
        prefetched[(0, 0)] = pf_it
        pf_it2 = itp.tile([128, 4, NT], bf16, tag="it")
        nc.sync.dma_start(pf_it2[:], imgT[0, :, :, NT:2 * NT])
        prefetched[(0, 1)] = pf_it2

        # ---- proj_hidden in [d, b] layout (per-partition bias for tanh) ----
        ph_ps = tpp.tile([128, 2, BPC], f32, tag="tp")
        for dh in range(2):
            for hc in range(4):
                nc.tensor.matmul(
                    ph_ps[:, dh, :],
                    lhsT=hw_sb[:, hc, dh * 128:(dh + 1) * 128],
                    rhs=hw_sb[:, hc, D:D + BPC],
                    start=(hc == 0),
                    stop=(hc == 3),
                )
        ph_sb = const.tile([128, 2, BPC], f32)
        nc.scalar.copy(ph_sb[:], ph_ps[:])

        sc_rows = {}   # group -> [128, 8, 512] f32 score rows (4 live rows)
        wn_bf = {}     # group -> [128, NCH, 128] bf16 transposed weights
        wn_fs = {}     # group -> [128, N] f32 normalized weights (for output)
        csbs = {}      # batch -> [1, F] f32 context rows (for output)

        def phase_a_steps(b):
            """proj + tanh + score rows for one batch; yields after each sub."""
            g, k = divmod(b, 4)
            if k == 0:
                srow_new = scrp.tile([128, NNT * NSUB, 512], f32, tag="scr")
                sc_rows[g] = srow_new
            srow = sc_rows[g]
            r = 32 * k
            pend = None  # (ths, chunk-col) for deferred scores matmuls

            def flush_scores():
                nonlocal pend
                if pend is None:
                    return
                ths, cc = pend
                sc = scp.tile([128, 512], f32, tag="sc")
                for dh in range(2):
                    nc.tensor.matmul(
                        sc[r:r + 1, :],
                        lhsT=wsc_sb[:, dh, :],
                        rhs=ths[dh][:],
                        start=(dh == 0),
                        stop=(dh == 1),
                        tile_position=(0, r),
                    )
                nc.vector.tensor_copy(srow[r:r + 1, cc, :], sc[r:r + 1, :])
                pend = None

            for nt in range(NNT):
                it = prefetched.pop((b, nt), None)
                if it is None:
                    it = itp.tile([128, 4, NT], bf16, tag="it")
                    nc.sync.dma_start(it[:], imgT[b, :, :, nt * NT:(nt + 1) * NT])
                for sub in range(NSUB):
                    sl = slice(sub * 512, (sub + 1) * 512)
                    ths = []
                    for dh in range(2):
                        pp = pps.tile([128, 512], f32, tag="pp")
                        for fc in range(4):
                            nc.tensor.matmul(
                                pp[:],
                                lhsT=wimg_sb[:, fc, dh * 128:(dh + 1) * 128],
                                rhs=it[:, fc, sl],
                                start=(fc == 0),
                                stop=(fc == 3),
                            )
                        th = thp.tile([128, 512], bf16, tag="th")
                        nc.scalar.activation(
                            th[:], pp[:], AF.Tanh,
                            bias=ph_sb[:, dh, b:b + 1], scale=1.0,
                        )
                        ths.append(th)
                    flush_scores()
                    pend = (ths, nt * NSUB + sub)
                    yield
            flush_scores()

        def softmax_group(g):
            """exp/sum/normalize/transpose for 4 batches at rows {0,32,64,96}."""
            srow = sc_rows[g]
            exp_f = exfp.tile([128, N], f32, tag="expf")
            se = smp.tile([128, 1], f32, tag="se")
            # exp with fused free-dim sum: one ACT op feeds both paths
            nc.scalar.activation(exp_f[:], srow[:].rearrange("p a b -> p (a b)"),
                                 AF.Exp, accum_out=se[:])
            rec = smp.tile([128, 1], f32, tag="rec")
            nc.vector.reciprocal(rec[:], se[:])
            wt = wtp.tile([128, NCH, 128], bf16, tag="wt")
            wn_bf[g] = wt
            # normalize + transpose per chunk so the first context matmul
            # unblocks early; transposes ride the otherwise-idle PE.
            for c in range(NCH):
                wn_c = smp.tile([128, 128], bf16, tag="wnc")
                nc.vector.tensor_scalar_mul(
                    wn_c[:], exp_f[:, c * 128:(c + 1) * 128], rec[:]
                )
                tp = tpp.tile([128, 128], bf16, tag="tp")
                nc.tensor.transpose(tp[:], wn_c[:], id_sb[:])
                nc.vector.tensor_copy(wt[:, c, :], tp[:])
            wn_f = wnfp.tile([128, N], f32, tag="wnf")
            nc.vector.tensor_scalar_mul(wn_f[:], exp_f[:], rec[:])
            wn_fs[g] = wn_f

        def phase_c_steps(g):
            """context for the 4 batches of group g, col-packed on the PE:
            the 4 M=1 matmuls per chunk land in distinct 32-col groups and
            execute concurrently in the array."""
            wt = wn_bf[g]
            cp = ctxps.tile([128, F], f32, tag="cp")
            for gg in range(NCH // CG):
                inats = []
                for k in range(4):
                    inat = inp.tile([128, CG, F], bf16, tag=f"in{k}")
                    nc.sync.dma_start(
                        inat[:], imgN[g * 4 + k, :, gg * CG:(gg + 1) * CG, :]
                    )
                    inats.append(inat)
                for j in range(CG):
                    c = gg * CG + j
                    for k in range(4):
                        nc.tensor.matmul(
                            cp[32 * k:32 * k + 1, :],
                            lhsT=wt[:, c, 32 * k:32 * k + 1],
                            rhs=inats[k][:, j, :],
                            start=(c == 0),
                            stop=(c == NCH - 1),
                            tile_position=(0, 32 * k),
                        )
                yield
            csb = csp.tile([128, F], f32, tag="csb")
            nc.vector.tensor_copy(csb[:], cp[:])
            csbs[g] = csb
            while True:
                yield

        # software pipeline: context(group 0) overlaps proj(group 1).
        # C(g0) has NCH//CG steps spread over the 32 A-subs of group 1.
        for b in range(4):
            for _ in phase_a_steps(b):
                pass
        softmax_group(0)
        cgen = phase_c_steps(0)
        csteps_left = NCH // CG + 1
        sub_i = 0
        for k in range(4):
            for _ in phase_a_steps(4 + k):
                sub_i += 1
                if sub_i % 4 == 0 and csteps_left > 0:
                    next(cgen)
                    csteps_left -= 1
        while csteps_left > 0:
            next(cgen)
            csteps_left -= 1
        softmax_group(1)
        cgen = phase_c_steps(1)
        for _ in range(NCH // CG + 1):
            next(cgen)

        # all output DMAs at the end (ACT ring; SP ring stays pure input)
        for g in range(2):
            for kk in range(4):
                nc.scalar.dma_start(wts_out[g * 4 + kk],
                                    wn_fs[g][32 * kk:32 * kk + 1, :])
                nc.scalar.dma_start(ctx_out[g * 4 + kk:g * 4 + kk + 1, :],
                                    csbs[g][32 * kk:32 * kk + 1, :])

    nc.compile()
    _nc_cache["nc"] = nc
    return nc


def _in_maps(image_features, hidden_state, W_img, W_hid, W_score):
    img_bf = image_features.astype(_BF)                        # [64, 4096, 512]
    # context rhs: [B, p, n-chunk, f] with f contiguous per (p, chunk)
    imgN_q = np.ascontiguousarray(
        img_bf.reshape(B, NCH, 128, F).transpose(0, 2, 1, 3)
    )                                                          # [64, 128, 32, 512]
    # proj rhs: [B, p, f-chunk, n] with n contiguous per (p, chunk)
    imgT_bf = img_bf.transpose(0, 2, 1)                        # [64, 512, 4096] view
    imgT_q = np.ascontiguousarray(
        imgT_bf.reshape(B, 4, 128, N).transpose(0, 2, 1, 3)
    )                                                          # [64, 128, 4, 4096]
    wimgT = np.ascontiguousarray(W_img.T).astype(_BF)          # [512, 256]
    whidT = W_hid.T.astype(np.float32)                         # [512, 256]
    wsc = np.ascontiguousarray(W_score.reshape(1, D).T).astype(_BF)  # [256, 1]
    eye = np.eye(128, dtype=np.float32).astype(_BF)

    in_maps = []
    for c in range(NCORES):
        s = slice(c * BPC, (c + 1) * BPC)
        hwpack = np.concatenate(
            [whidT, hidden_state[s].T.astype(np.float32)], axis=1
        )  # [512, 264]
        in_maps.append({
            "imgT": imgT_q[s],
            "imgN": imgN_q[s],
            "hw": np.ascontiguousarray(hwpack),
            "wimgT": wimgT,
            "wscore": wsc,
            "ident": eye,
        })
    return in_maps


def kernel(image_features, hidden_state, W_img, W_hid, W_score):
    from concourse.bass_utils import run_bass_kernel_spmd

    nc = _build_nc()
    in_maps = _in_maps(image_features, hidden_state, W_img, W_hid, W_score)
    res = run_bass_kernel_spmd(nc, in_maps, list(range(NCORES))).results
    ctx = np.concatenate([r["ctx"] for r in res], axis=0)
    wts = np.concatenate([r["wts"] for r in res], axis=0)
    return (ctx, wts)
